# revision 5
# baseline (speedup 1.0000x reference)
"""Chamfer loss kernel for Trainium2, 8 NeuronCores.

Strategy (sharding_hint): row-block the 16384x16384 distance matrix.
Core c owns x rows [c*2048, (c+1)*2048) (x = flattened pred corners) and
all 16384 y points (flattened gt corners). Each core computes, on device:
  - d2[n, m] = |x_n|^2 + |y_m|^2 - 2 x.y  for its row block, via a K=16
    fp16 matmul using hi/lo fp16 splits of the operands (~fp32 accuracy).
    The PE array is row-tiled 4x (tile_position=(32c,0)): row group c
    handles the 4096-column chunk c, so 4 matmuls stream concurrently in
    the 128x128 array (K=16 uses only 16 of 32 rows per group).
  - per-PSUM-tile drain to SBUF fp16 on the scalar (ACT) engine,
  - row mins via a single fused tensor_tensor_reduce (halves fold + min
    accumulate) per x-tile on the DVE,
  - a running column min (TT-min) per x-tile on the DVE,
  - column partition-min via PE transposes + tensor_reduce epilogue.
Host glue: shard x, gather per-core row mins, all-reduce(min) the partial
column mins across the 8 cores, then mean both and add - the loss.
"""

import sys
import numpy as np

if "/opt/trn_rl_repo" not in sys.path:
    sys.path.insert(0, "/opt/trn_rl_repo")

# ---- hardcoded problem geometry (from the task spec) ----
N_CORES = 8
NX = 16384          # total x points (2048 boxes * 8 corners)
NY = 16384          # total y points
RP = NX // N_CORES  # 2048 x rows per core
XT = RP // 128      # 16 x tiles of 128 rows
K = 16              # contraction rows of the split matmul
NGRP = 4            # PE row groups (4x tiling); group c owns cols [c*4096, +4096)
CHUNK = NY // NGRP  # 4096 cols per row group
DR = 2048           # columns per drain tile (4 PSUM banks)
NDR = NY // DR      # 8 drain steps per x-tile


def build_module(rp=RP, ny=NY, n_grp=NGRP, use_ttr=False):
    # NOTE: use_ttr=True (InstTensorTensorReduce) crashes on TRN2 hardware
    # (and is 1x-rate anyway); keep the TT-min halving tree.
    """Build + compile the per-core Bass module. Returns the Bacc object.

    n_grp: number of PE row groups (4 = full row tiling, 1 = none).
    use_ttr: row-min via fused tensor_tensor_reduce vs TT-min tree.
    """
    from contextlib import ExitStack

    import concourse.tile as tile
    from concourse import bacc, mybir
    from concourse.masks import make_identity

    fp32 = mybir.dt.float32
    fp16 = mybir.dt.float16
    AX = mybir.AxisListType
    OP = mybir.AluOpType
    ACT = mybir.ActivationFunctionType

    xt_n = rp // 128
    xf = rp // 128       # free cols per partition for x feature tiles
    yf = ny // 128
    chunk = ny // n_grp  # columns per PE row group
    ndr = ny // DR       # drain steps per x-tile

    nc = bacc.Bacc("TRN2", target_bir_lowering=False, debug=False,
                   num_devices=N_CORES)
    x_h = nc.dram_tensor("x_shard", [rp, 3], fp32, kind="ExternalInput")
    y_h = nc.dram_tensor("y_full", [ny, 3], fp32, kind="ExternalInput")
    row_h = nc.dram_tensor("row_out", [128, xt_n], fp32, kind="ExternalOutput")
    col_h = nc.dram_tensor("col_out", [128, yf], fp32, kind="ExternalOutput")

    with tile.TileContext(nc) as tc:
        with ExitStack() as ctx:
            const_pool = ctx.enter_context(tc.tile_pool(name="const", bufs=1))
            prep_pool = ctx.enter_context(tc.tile_pool(name="prep", bufs=1))
            big_pool = ctx.enter_context(tc.tile_pool(name="big", bufs=1))
            dst_pool = ctx.enter_context(tc.tile_pool(name="dst", bufs=2))
            out_pool = ctx.enter_context(tc.tile_pool(name="outp", bufs=1))

            # ---------- constants ----------
            ones_y = const_pool.tile([128, yf], fp16, tag="ones_y")
            nc.vector.memset(ones_y[:], 1.0)
            ident = const_pool.tile([128, 128], fp16, tag="ident")
            make_identity(nc, ident[:])

            # ---------- feature prep: y ----------
            # cy[p, d*yf + f] = y[p*yf + f, d]
            # one contiguous DMA (fast), then de-interleave xyz on the DVE -
            # the 4B/12B strided DRAM read pattern costs ~14us per plane.
            craw_y = prep_pool.tile([128, 3 * yf], fp32, tag="craw_y")
            nc.sync.dma_start(
                craw_y[:], y_h.ap().rearrange("(p f) d -> p (f d)", p=128))
            cy = prep_pool.tile([128, 3 * yf], fp32, tag="cy")
            craw_y3 = craw_y[:].rearrange("p (f d) -> p d f", d=3)
            for d in range(3):
                nc.vector.tensor_copy(cy[:, d * yf:(d + 1) * yf],
                                      craw_y3[:, d:d + 1, :])
            n2y = prep_pool.tile([128, yf], fp32, tag="n2y")
            tmpy = prep_pool.tile([128, yf], fp32, tag="tmpy")
            nc.vector.tensor_tensor(n2y[:], cy[:, 0:yf], cy[:, 0:yf], op=OP.mult)
            nc.vector.tensor_tensor(tmpy[:], cy[:, yf:2 * yf], cy[:, yf:2 * yf], op=OP.mult)
            nc.vector.tensor_tensor(n2y[:], n2y[:], tmpy[:], op=OP.add)
            nc.vector.tensor_tensor(tmpy[:], cy[:, 2 * yf:3 * yf], cy[:, 2 * yf:3 * yf], op=OP.mult)
            nc.vector.tensor_tensor(n2y[:], n2y[:], tmpy[:], op=OP.add)
            # hi/lo split of n2y
            n2yh = prep_pool.tile([128, yf], fp16, tag="n2yh")
            n2yh32 = prep_pool.tile([128, yf], fp32, tag="n2yh32")
            n2yl = prep_pool.tile([128, yf], fp16, tag="n2yl")
            nc.vector.tensor_copy(n2yh[:], n2y[:])
            nc.scalar.copy(n2yh32[:], n2yh[:])
            nc.vector.tensor_tensor(n2yl[:], n2y[:], n2yh32[:], op=OP.subtract)
            # hi/lo split of y coords (all 3 at once)
            yh = prep_pool.tile([128, 3 * yf], fp16, tag="yh")
            yh32 = prep_pool.tile([128, 3 * yf], fp32, tag="yh32")
            yl = prep_pool.tile([128, 3 * yf], fp16, tag="yl")
            nc.vector.tensor_copy(yh[:], cy[:])
            nc.scalar.copy(yh32[:], yh[:])
            nc.vector.tensor_tensor(yl[:], cy[:], yh32[:], op=OP.subtract)

            # ---------- feature prep: x ----------
            craw_x = prep_pool.tile([128, 3 * xf], fp32, tag="craw_x")
            nc.sync.dma_start(
                craw_x[:], x_h.ap().rearrange("(p f) d -> p (f d)", p=128))
            cx = prep_pool.tile([128, 3 * xf], fp32, tag="cx")
            craw_x3 = craw_x[:].rearrange("p (f d) -> p d f", d=3)
            for d in range(3):
                nc.vector.tensor_copy(cx[:, d * xf:(d + 1) * xf],
                                      craw_x3[:, d:d + 1, :])
            n2x = prep_pool.tile([128, xf], fp32, tag="n2x")
            tmpx = prep_pool.tile([128, xf], fp32, tag="tmpx")
            nc.vector.tensor_tensor(n2x[:], cx[:, 0:xf], cx[:, 0:xf], op=OP.mult)
            nc.vector.tensor_tensor(tmpx[:], cx[:, xf:2 * xf], cx[:, xf:2 * xf], op=OP.mult)
            nc.vector.tensor_tensor(n2x[:], n2x[:], tmpx[:], op=OP.add)
            nc.vector.tensor_tensor(tmpx[:], cx[:, 2 * xf:3 * xf], cx[:, 2 * xf:3 * xf], op=OP.mult)
            nc.vector.tensor_tensor(n2x[:], n2x[:], tmpx[:], op=OP.add)
            n2xh = prep_pool.tile([128, xf], fp16, tag="n2xh")
            n2xh32 = prep_pool.tile([128, xf], fp32, tag="n2xh32")
            n2xl = prep_pool.tile([128, xf], fp16, tag="n2xl")
            nc.vector.tensor_copy(n2xh[:], n2x[:])
            nc.scalar.copy(n2xh32[:], n2xh[:])
            nc.vector.tensor_tensor(n2xl[:], n2x[:], n2xh32[:], op=OP.subtract)
            # a = -2x, then hi/lo split
            ax = prep_pool.tile([128, 3 * xf], fp32, tag="ax")
            nc.vector.tensor_scalar_mul(ax[:], cx[:], -2.0)
            axh = prep_pool.tile([128, 3 * xf], fp16, tag="axh")
            axh32 = prep_pool.tile([128, 3 * xf], fp32, tag="axh32")
            axl = prep_pool.tile([128, 3 * xf], fp16, tag="axl")
            nc.vector.tensor_copy(axh[:], ax[:])
            nc.scalar.copy(axh32[:], axh[:])
            nc.vector.tensor_tensor(axl[:], ax[:], axh32[:], op=OP.subtract)

            # ---------- assemble K x N operand tiles ----------
            # pairing per K row r:  phi[r] . psi[r]
            #  r0 : 1      * |y|2_h     r1 : 1      * |y|2_l
            #  r2 : |x|2_h * 1          r3 : |x|2_l * 1
            #  r4..6  : axh_d * yh_d    r7..9  : axh_d * yl_d
            #  r10..12: axl_d * yh_d    r13..15: axl_d * yl_d
            # Round-trip through DRAM scratch: the SBUF->DRAM writes keep the
            # [128, f] layout (768B/partition descriptors), and each psi/phi
            # row read becomes one small 2D strided DRAM read - far cheaper
            # than a [128-partition gather] -> [1 partition] SBUF-SBUF DMA.
            # For the 4x PE row tiling, row k of PE group c lives on SBUF
            # partition 32c+k: phi rows are replicated to all 4 bases, psi
            # rows are split by column chunk (group c gets chunk c).
            dram_pool = ctx.enter_context(
                tc.tile_pool(name="dscr", bufs=1, space="DRAM"))
            d_yh = dram_pool.tile([128, 3 * yf], fp16, tag="d_yh")
            d_yl = dram_pool.tile([128, 3 * yf], fp16, tag="d_yl")
            d_n2yh = dram_pool.tile([128, yf], fp16, tag="d_n2yh")
            d_n2yl = dram_pool.tile([128, yf], fp16, tag="d_n2yl")
            d_ones = dram_pool.tile([128, yf], fp16, tag="d_ones")
            d_xh = dram_pool.tile([128, 3 * xf], fp16, tag="d_xh")
            d_xl = dram_pool.tile([128, 3 * xf], fp16, tag="d_xl")
            d_n2xh = dram_pool.tile([128, xf], fp16, tag="d_n2xh")
            d_n2xl = dram_pool.tile([128, xf], fp16, tag="d_n2xl")
            nc.sync.dma_start(d_yh[:], yh[:])
            nc.sync.dma_start(d_yl[:], yl[:])
            nc.sync.dma_start(d_n2yh[:], n2yh[:])
            nc.sync.dma_start(d_n2yl[:], n2yl[:])
            nc.sync.dma_start(d_ones[:], ones_y[:])
            nc.sync.dma_start(d_xh[:], axh[:])
            nc.sync.dma_start(d_xl[:], axl[:])
            nc.sync.dma_start(d_n2xh[:], n2xh[:])
            nc.sync.dma_start(d_n2xl[:], n2xl[:])

            # phi_rep[32c + k, :] = phi row k (same for all c)
            # psi_rep[32c + k, :] = psi row k, global columns [c*4096, +4096)
            phi_rep = big_pool.tile([128, rp], fp16, tag="phi_rep")
            psi_rep = big_pool.tile([128, chunk], fp16, tag="psi_rep")

            _eng = [nc.sync, nc.gpsimd, nc.scalar]
            _rr = [0]

            def dma(dst, src):
                e = _eng[_rr[0] % len(_eng)]
                _rr[0] += 1
                e.dma_start(dst, src)

            hp = chunk // yf   # partitions of a [128, yf] dram tile per chunk

            def flat(t, c):    # [128, yf] dram tile -> linear chunk-c row
                return t[c * hp:(c + 1) * hp, :].rearrange("p f -> (p f)")

            def plane(t, d, c):  # [128, 3f] d-major dram tile -> coord chunk c
                return t[:].rearrange(
                    "p (d f) -> d p f", d=3)[d:d + 1, c * hp:(c + 1) * hp, :]

            def plane3(t, d):    # full x coord row
                return t[:].rearrange("p (d f) -> d p f", d=3)[d:d + 1, :, :]

            ones_rp = d_ones[0:rp // yf, :].rearrange("p f -> (p f)")
            for c in range(n_grp):
                b = 32 * c

                def ph(k):
                    return phi_rep[b + k:b + k + 1, :]

                dma(ph(0), ones_rp)
                dma(ph(1), ones_rp)
                dma(ph(2), d_n2xh[:, :].rearrange("p f -> (p f)"))
                dma(ph(3), d_n2xl[:, :].rearrange("p f -> (p f)"))
                for d in range(3):
                    dma(ph(4 + d), plane3(d_xh, d))
                    dma(ph(7 + d), plane3(d_xh, d))
                    dma(ph(10 + d), plane3(d_xl, d))
                    dma(ph(13 + d), plane3(d_xl, d))

            # psi chunks in group order so group 0's matmuls can start early
            for c in range(n_grp):
                b = 32 * c

                def ps(k):
                    return psi_rep[b + k:b + k + 1, :]

                dma(ps(0), flat(d_n2yh, c))
                dma(ps(1), flat(d_n2yl, c))
                dma(ps(2), flat(d_ones, c))
                dma(ps(3), flat(d_ones, c))
                for d in range(3):
                    dma(ps(4 + d), plane(d_yh, d, c))
                    dma(ps(7 + d), plane(d_yl, d, c))
                    dma(ps(10 + d), plane(d_yh, d, c))
                    dma(ps(13 + d), plane(d_yl, d, c))

            # ---------- main loop ----------
            # Per x-tile: 8 drain steps r; step r is PE row group c=r%4,
            # chunk-half h=r//4, i.e. dst cols [r*2048, +2048) = global cols
            # [c*4096 + h*2048, +2048)  (a fixed column permutation - min and
            # mean are permutation invariant, and all cores use the same one).
            colacc = big_pool.tile([128, ny], fp16, tag="colacc")
            fold = big_pool.tile([128, ny // 2], fp16, tag="fold")
            rmin = out_pool.tile([128, xt_n], fp32, tag="rmin")

            with tc.tile_pool(name="psum", bufs=2, space="PSUM") as psum_pool:
                for xt in range(xt_n):
                    dst = colacc if xt == 0 else dst_pool.tile(
                        [128, ny], fp16, tag="dst")
                    for r in range(ndr):
                        c = r % n_grp
                        h = r // n_grp
                        b = 32 * c
                        w = phi_rep[b:b + K, xt * 128:(xt + 1) * 128]
                        pt = psum_pool.tile([128, DR], fp32, tag="pt")
                        for q in range(DR // 512):
                            col = h * DR + q * 512
                            nc.tensor.matmul(
                                pt[:, q * 512:(q + 1) * 512],
                                w, psi_rep[b:b + K, col:col + 512],
                                start=True, stop=True,
                                tile_position=(b, 0),
                            )
                        nc.scalar.copy(dst[:, r * DR:(r + 1) * DR], pt[:])
                    if use_ttr:
                        # fused fold + row-min: fold = min(dstL, dstR),
                        # rmin[:, xt] = min over the fold (whole row).
                        nc.vector.tensor_tensor_reduce(
                            out=fold[:],
                            in0=dst[:, :ny // 2],
                            in1=dst[:, ny // 2:],
                            scale=1.0,
                            scalar=1.0e30,
                            op0=OP.min,
                            op1=OP.min,
                            accum_out=rmin[:, xt:xt + 1],
                        )
                    else:
                        nc.vector.tensor_tensor(
                            fold[:], dst[:, :ny // 2], dst[:, ny // 2:],
                            op=OP.min)
                        w2 = ny // 4
                        while w2 >= 256:
                            nc.vector.tensor_tensor(
                                fold[:, :w2], fold[:, :w2], fold[:, w2:2 * w2],
                                op=OP.min)
                            w2 //= 2
                        nc.vector.tensor_reduce(
                            rmin[:, xt:xt + 1], fold[:, :256], axis=AX.X,
                            op=OP.min)
                    if xt == xt_n - 1:
                        # chunk the last col-min update so the epilogue's
                        # transposes can start underneath it
                        for cch in range(4):
                            sl = slice(cch * ny // 4, (cch + 1) * ny // 4)
                            nc.vector.tensor_tensor(
                                colacc[:, sl], colacc[:, sl], dst[:, sl],
                                op=OP.min)
                    elif xt > 0:
                        nc.vector.tensor_tensor(
                            colacc[:], colacc[:], dst[:], op=OP.min)

            # ---------- epilogue: partition-min of colacc via PE transpose ----------
            colmin16 = out_pool.tile([128, yf], fp16, tag="colmin16")
            with tc.tile_pool(name="psumT", bufs=2, space="PSUM") as psumt_pool:
                bb = 16  # transposed blocks per batch
                nb = yf // bb
                for b in range(nb):
                    ptile = psumt_pool.tile([128, bb * 128], fp16, tag="ptile")
                    for q in range(bb):
                        blk = b * bb + q
                        nc.tensor.transpose(
                            ptile[:, q * 128:(q + 1) * 128],
                            colacc[:, blk * 128:(blk + 1) * 128],
                            ident[:],
                        )
                    nc.vector.tensor_reduce(
                        colmin16[:, b * bb:(b + 1) * bb],
                        ptile[:].rearrange("p (a f) -> p a f", a=bb),
                        axis=AX.X, op=OP.min,
                    )

            # ---------- clamp + sqrt + store ----------
            colmin32 = out_pool.tile([128, yf], fp32, tag="colmin32")
            nc.vector.tensor_scalar_max(colmin32[:], colmin16[:], 0.0)
            colout = out_pool.tile([128, yf], fp32, tag="colout")
            nc.scalar.activation(colout[:], colmin32[:], ACT.Sqrt)
            nc.sync.dma_start(col_h.ap()[:, :], colout[:])

            rclamp = out_pool.tile([128, xt_n], fp32, tag="rclamp")
            nc.vector.tensor_scalar_max(rclamp[:], rmin[:], 0.0)
            rowout = out_pool.tile([128, xt_n], fp32, tag="rowout")
            nc.scalar.activation(rowout[:], rclamp[:], ACT.Sqrt)
            nc.sync.dma_start(row_h.ap()[:, :], rowout[:])

    nc.compile()
    return nc


_CACHED = None


def _get_module():
    global _CACHED
    if _CACHED is None:
        _CACHED = build_module()
    return _CACHED


def run_on_hw(nc, in_maps, **kw):
    from concourse.bass_utils import run_bass_kernel_spmd
    return run_bass_kernel_spmd(nc, in_maps, core_ids=list(range(N_CORES)), **kw)


def _postprocess(results):
    rowcat = np.concatenate(
        [results[c]["row_out"].T.reshape(-1) for c in range(N_CORES)])
    colmin = np.stack(
        [results[c]["col_out"].T.reshape(-1) for c in range(N_CORES)]).min(axis=0)
    loss = rowcat.mean(dtype=np.float64) + colmin.mean(dtype=np.float64)
    return np.asarray(loss, dtype=np.float32)


def kernel(pred_corners, gt_corners):
    x = np.ascontiguousarray(np.asarray(pred_corners, dtype=np.float32).reshape(-1, 3))
    y = np.ascontiguousarray(np.asarray(gt_corners, dtype=np.float32).reshape(-1, 3))
    assert x.shape == (NX, 3) and y.shape == (NY, 3)
    nc = _get_module()
    in_maps = [
        {"x_shard": x[c * RP:(c + 1) * RP], "y_full": y} for c in range(N_CORES)
    ]
    res = run_on_hw(nc, in_maps)
    return _postprocess(res.results)


# revision 10
# speedup vs baseline: 1.0409x; 1.0409x over previous
"""Chamfer loss kernel for Trainium2, 8 NeuronCores.

Strategy (sharding_hint): row-block the 16384x16384 distance matrix.
Core c owns x rows [c*2048, (c+1)*2048) (x = flattened pred corners) and
all 16384 y points (flattened gt corners). Each core computes, on device:
  - d2[n, m] = |x_n|^2 + |y_m|^2 - 2 x.y  for its row block, via a K=16
    fp16 matmul using hi/lo fp16 splits of the operands (~fp32 accuracy).
    The PE array is row-tiled 4x (tile_position=(32c,0)): row group c
    handles the 4096-column chunk c, so 4 matmuls stream concurrently in
    the 128x128 array (K=16 uses only 16 of 32 rows per group).
  - per-PSUM-tile drain to SBUF fp16 on the scalar (ACT) engine,
  - row mins via a single fused tensor_tensor_reduce (halves fold + min
    accumulate) per x-tile on the DVE,
  - a running column min (TT-min) per x-tile on the DVE,
  - column partition-min via PE transposes + tensor_reduce epilogue.
Host glue: shard x, gather per-core row mins, all-reduce(min) the partial
column mins across the 8 cores, then mean both and add - the loss.
"""

import sys
import numpy as np

if "/opt/trn_rl_repo" not in sys.path:
    sys.path.insert(0, "/opt/trn_rl_repo")

# ---- hardcoded problem geometry (from the task spec) ----
N_CORES = 8
NX = 16384          # total x points (2048 boxes * 8 corners)
NY = 16384          # total y points
RP = NX // N_CORES  # 2048 x rows per core
XT = RP // 128      # 16 x tiles of 128 rows
K = 16              # contraction rows of the split matmul
NGRP = 4            # PE row groups (4x tiling); group c owns cols [c*4096, +4096)
CHUNK = NY // NGRP  # 4096 cols per row group
DR = 2048           # columns per drain tile (4 PSUM banks)
NDR = NY // DR      # 8 drain steps per x-tile


def build_module(rp=RP, ny=NY, n_grp=NGRP, use_ttr=False, gp_folds=False):
    # NOTE: gp_folds=True (nc.gpsimd.tensor_tensor fp16 min) also crashes on
    # TRN2 hardware via this runner; row-tree stays on the DVE.
    # NOTE: use_ttr=True (InstTensorTensorReduce) crashes on TRN2 hardware
    # (and is 1x-rate anyway); keep the TT-min halving tree.
    """Build + compile the per-core Bass module. Returns the Bacc object.

    n_grp: number of PE row groups (4 = full row tiling, 1 = none).
    use_ttr: row-min via fused tensor_tensor_reduce vs TT-min tree.
    """
    from contextlib import ExitStack

    import concourse.tile as tile
    from concourse import bacc, mybir
    from concourse.masks import make_identity

    fp32 = mybir.dt.float32
    fp16 = mybir.dt.float16
    AX = mybir.AxisListType
    OP = mybir.AluOpType
    ACT = mybir.ActivationFunctionType

    xt_n = rp // 128
    xf = rp // 128       # free cols per partition for x feature tiles
    yf = ny // 128
    chunk = ny // n_grp  # columns per PE row group
    ndr = ny // DR       # drain steps per x-tile

    nc = bacc.Bacc("TRN2", target_bir_lowering=False, debug=False,
                   num_devices=N_CORES)
    x_h = nc.dram_tensor("x_shard", [rp, 3], fp32, kind="ExternalInput")
    y_h = nc.dram_tensor("y_full", [ny, 3], fp32, kind="ExternalInput")
    row_h = nc.dram_tensor("row_out", [128, xt_n], fp32, kind="ExternalOutput")
    col_h = nc.dram_tensor("col_out", [128, yf], fp32, kind="ExternalOutput")

    with tile.TileContext(nc) as tc:
        with ExitStack() as ctx:
            const_pool = ctx.enter_context(tc.tile_pool(name="const", bufs=1))
            prep_pool = ctx.enter_context(tc.tile_pool(name="prep", bufs=1))
            big_pool = ctx.enter_context(tc.tile_pool(name="big", bufs=1))
            dst_pool = ctx.enter_context(tc.tile_pool(name="dst", bufs=2))
            fold_pool = ctx.enter_context(tc.tile_pool(name="fold", bufs=2))
            out_pool = ctx.enter_context(tc.tile_pool(name="outp", bufs=1))

            # ---------- constants ----------
            ones_y = const_pool.tile([128, yf], fp16, tag="ones_y")
            nc.vector.memset(ones_y[:], 1.0)
            ident = const_pool.tile([128, 128], fp16, tag="ident")
            make_identity(nc, ident[:])

            # ---------- feature prep: y ----------
            # cy[p, d*yf + f] = y[p*yf + f, d]
            # one contiguous DMA (fast), then de-interleave xyz on the DVE -
            # the 4B/12B strided DRAM read pattern costs ~14us per plane.
            craw_y = prep_pool.tile([128, 3 * yf], fp32, tag="craw_y")
            nc.sync.dma_start(
                craw_y[:], y_h.ap().rearrange("(p f) d -> p (f d)", p=128))
            cy = prep_pool.tile([128, 3 * yf], fp32, tag="cy")
            craw_y3 = craw_y[:].rearrange("p (f d) -> p d f", d=3)
            for d in range(3):
                nc.vector.tensor_copy(cy[:, d * yf:(d + 1) * yf],
                                      craw_y3[:, d:d + 1, :])
            n2y = prep_pool.tile([128, yf], fp32, tag="n2y")
            tmpy = prep_pool.tile([128, yf], fp32, tag="tmpy")
            nc.vector.tensor_tensor(n2y[:], cy[:, 0:yf], cy[:, 0:yf], op=OP.mult)
            nc.vector.tensor_tensor(tmpy[:], cy[:, yf:2 * yf], cy[:, yf:2 * yf], op=OP.mult)
            nc.vector.tensor_tensor(n2y[:], n2y[:], tmpy[:], op=OP.add)
            nc.vector.tensor_tensor(tmpy[:], cy[:, 2 * yf:3 * yf], cy[:, 2 * yf:3 * yf], op=OP.mult)
            nc.vector.tensor_tensor(n2y[:], n2y[:], tmpy[:], op=OP.add)
            # hi/lo split of n2y
            n2yh = prep_pool.tile([128, yf], fp16, tag="n2yh")
            n2yh32 = prep_pool.tile([128, yf], fp32, tag="n2yh32")
            n2yl = prep_pool.tile([128, yf], fp16, tag="n2yl")
            nc.vector.tensor_copy(n2yh[:], n2y[:])
            nc.scalar.copy(n2yh32[:], n2yh[:])
            nc.vector.tensor_tensor(n2yl[:], n2y[:], n2yh32[:], op=OP.subtract)
            # hi/lo split of y coords (all 3 at once)
            yh = prep_pool.tile([128, 3 * yf], fp16, tag="yh")
            yh32 = prep_pool.tile([128, 3 * yf], fp32, tag="yh32")
            yl = prep_pool.tile([128, 3 * yf], fp16, tag="yl")
            nc.vector.tensor_copy(yh[:], cy[:])
            nc.scalar.copy(yh32[:], yh[:])
            nc.vector.tensor_tensor(yl[:], cy[:], yh32[:], op=OP.subtract)

            # ---------- feature prep: x ----------
            craw_x = prep_pool.tile([128, 3 * xf], fp32, tag="craw_x")
            nc.sync.dma_start(
                craw_x[:], x_h.ap().rearrange("(p f) d -> p (f d)", p=128))
            cx = prep_pool.tile([128, 3 * xf], fp32, tag="cx")
            craw_x3 = craw_x[:].rearrange("p (f d) -> p d f", d=3)
            for d in range(3):
                nc.vector.tensor_copy(cx[:, d * xf:(d + 1) * xf],
                                      craw_x3[:, d:d + 1, :])
            n2x = prep_pool.tile([128, xf], fp32, tag="n2x")
            tmpx = prep_pool.tile([128, xf], fp32, tag="tmpx")
            nc.vector.tensor_tensor(n2x[:], cx[:, 0:xf], cx[:, 0:xf], op=OP.mult)
            nc.vector.tensor_tensor(tmpx[:], cx[:, xf:2 * xf], cx[:, xf:2 * xf], op=OP.mult)
            nc.vector.tensor_tensor(n2x[:], n2x[:], tmpx[:], op=OP.add)
            nc.vector.tensor_tensor(tmpx[:], cx[:, 2 * xf:3 * xf], cx[:, 2 * xf:3 * xf], op=OP.mult)
            nc.vector.tensor_tensor(n2x[:], n2x[:], tmpx[:], op=OP.add)
            n2xh = prep_pool.tile([128, xf], fp16, tag="n2xh")
            n2xh32 = prep_pool.tile([128, xf], fp32, tag="n2xh32")
            n2xl = prep_pool.tile([128, xf], fp16, tag="n2xl")
            nc.vector.tensor_copy(n2xh[:], n2x[:])
            nc.scalar.copy(n2xh32[:], n2xh[:])
            nc.vector.tensor_tensor(n2xl[:], n2x[:], n2xh32[:], op=OP.subtract)
            # a = -2x, then hi/lo split
            ax = prep_pool.tile([128, 3 * xf], fp32, tag="ax")
            nc.vector.tensor_scalar_mul(ax[:], cx[:], -2.0)
            axh = prep_pool.tile([128, 3 * xf], fp16, tag="axh")
            axh32 = prep_pool.tile([128, 3 * xf], fp32, tag="axh32")
            axl = prep_pool.tile([128, 3 * xf], fp16, tag="axl")
            nc.vector.tensor_copy(axh[:], ax[:])
            nc.scalar.copy(axh32[:], axh[:])
            nc.vector.tensor_tensor(axl[:], ax[:], axh32[:], op=OP.subtract)

            # ---------- assemble K x N operand tiles ----------
            # pairing per K row r:  phi[r] . psi[r]
            #  r0 : 1      * |y|2_h     r1 : 1      * |y|2_l
            #  r2 : |x|2_h * 1          r3 : |x|2_l * 1
            #  r4..6  : axh_d * yh_d    r7..9  : axh_d * yl_d
            #  r10..12: axl_d * yh_d    r13..15: axl_d * yl_d
            # Round-trip through DRAM scratch: the SBUF->DRAM writes keep the
            # [128, f] layout (768B/partition descriptors), and each psi/phi
            # row read becomes one small 2D strided DRAM read - far cheaper
            # than a [128-partition gather] -> [1 partition] SBUF-SBUF DMA.
            # For the 4x PE row tiling, row k of PE group c lives on SBUF
            # partition 32c+k: phi rows are replicated to all 4 bases, psi
            # rows are split by column chunk (group c gets chunk c).
            dram_pool = ctx.enter_context(
                tc.tile_pool(name="dscr", bufs=1, space="DRAM"))
            d_yh = dram_pool.tile([128, 3 * yf], fp16, tag="d_yh")
            d_yl = dram_pool.tile([128, 3 * yf], fp16, tag="d_yl")
            d_n2yh = dram_pool.tile([128, yf], fp16, tag="d_n2yh")
            d_n2yl = dram_pool.tile([128, yf], fp16, tag="d_n2yl")
            d_ones = dram_pool.tile([128, yf], fp16, tag="d_ones")
            d_xh = dram_pool.tile([128, 3 * xf], fp16, tag="d_xh")
            d_xl = dram_pool.tile([128, 3 * xf], fp16, tag="d_xl")
            d_n2xh = dram_pool.tile([128, xf], fp16, tag="d_n2xh")
            d_n2xl = dram_pool.tile([128, xf], fp16, tag="d_n2xl")
            nc.sync.dma_start(d_yh[:], yh[:])
            nc.sync.dma_start(d_yl[:], yl[:])
            nc.sync.dma_start(d_n2yh[:], n2yh[:])
            nc.sync.dma_start(d_n2yl[:], n2yl[:])
            nc.sync.dma_start(d_ones[:], ones_y[:])
            nc.sync.dma_start(d_xh[:], axh[:])
            nc.sync.dma_start(d_xl[:], axl[:])
            nc.sync.dma_start(d_n2xh[:], n2xh[:])
            nc.sync.dma_start(d_n2xl[:], n2xl[:])

            # phi_rep[32c + k, :] = phi row k (same for all c)
            # psi_rep[32c + k, :] = psi row k, global columns [c*4096, +4096)
            # Assemble both at base 0 first (one strided-DRAM read per row),
            # then replicate/shift to bases 32/64/96 with a few fat
            # SBUF->SBUF DMAs (16 partitions x contiguous bytes each).
            phi_rep = big_pool.tile([128, rp], fp16, tag="phi_rep")
            psi_mst = big_pool.tile([128, ny], fp16, tag="psi_mst")
            psi_rep = big_pool.tile([128, chunk], fp16, tag="psi_rep")

            _eng = [nc.sync, nc.gpsimd, nc.scalar]
            _rr = [0]

            def dma(dst, src):
                e = _eng[_rr[0] % len(_eng)]
                _rr[0] += 1
                e.dma_start(dst, src)

            def flat(t):       # [128, yf] dram tile -> full linear row
                return t[:, :].rearrange("p f -> (p f)")

            def plane3(t, d):  # [128, 3f] d-major dram tile -> full coord row
                return t[:].rearrange("p (d f) -> d p f", d=3)[d:d + 1, :, :]

            ones_rp = d_ones[0:rp // yf, :].rearrange("p f -> (p f)")

            def ph(k):
                return phi_rep[k:k + 1, :]

            def ps(k):
                return psi_mst[k:k + 1, :]

            dma(ph(0), ones_rp)
            dma(ph(1), ones_rp)
            dma(ph(2), d_n2xh[:, :].rearrange("p f -> (p f)"))
            dma(ph(3), d_n2xl[:, :].rearrange("p f -> (p f)"))
            for d in range(3):
                dma(ph(4 + d), plane3(d_xh, d))
                dma(ph(7 + d), plane3(d_xh, d))
                dma(ph(10 + d), plane3(d_xl, d))
                dma(ph(13 + d), plane3(d_xl, d))

            dma(ps(0), flat(d_n2yh))
            dma(ps(1), flat(d_n2yl))
            dma(ps(2), flat(d_ones))
            dma(ps(3), flat(d_ones))
            for d in range(3):
                dma(ps(4 + d), plane3(d_yh, d))
                dma(ps(7 + d), plane3(d_yl, d))
                dma(ps(10 + d), plane3(d_yh, d))
                dma(ps(13 + d), plane3(d_yl, d))

            # replicate phi to bases 32/64/96; shift psi chunks c>=1 there.
            # group 0 reads psi_mst / phi_rep base 0 directly.
            for c in range(1, n_grp):
                nc.sync.dma_start(phi_rep[32 * c:32 * c + K, :],
                                  phi_rep[0:K, :])
                nc.gpsimd.dma_start(
                    psi_rep[32 * c:32 * c + K, :],
                    psi_mst[0:K, c * chunk:(c + 1) * chunk])

            # ---------- main loop ----------
            # Per x-tile: 8 drain steps r; step r is PE row group c=r%4,
            # chunk-half h=r//4, i.e. dst cols [r*2048, +2048) = global cols
            # [c*4096 + h*2048, +2048)  (a fixed column permutation - min and
            # mean are permutation invariant, and all cores use the same one).
            colacc = big_pool.tile([128, ny], fp16, tag="colacc")
            rmin = out_pool.tile([128, xt_n], fp32, tag="rmin")

            with tc.tile_pool(name="psum", bufs=2, space="PSUM") as psum_pool:
                for xt in range(xt_n):
                    dst = colacc if xt == 0 else dst_pool.tile(
                        [128, ny], fp16, tag="dst")
                    fold = fold_pool.tile([128, ny // 2], fp16, tag="fold")
                    for t in range(ndr):
                        # psum tile t: column c*512+j of the tile holds
                        # global column c*4096 + t*512 + j -> the 4 matmuls
                        # hit 4 different PE row groups and run concurrently.
                        pt = psum_pool.tile([128, DR], fp32, tag="pt")
                        for c in range(n_grp):
                            b = 32 * c
                            w = phi_rep[b:b + K, xt * 128:(xt + 1) * 128]
                            src_ps = psi_mst if c == 0 else psi_rep
                            rhs = src_ps[b:b + K, t * 512:(t + 1) * 512]
                            nc.tensor.matmul(
                                pt[:, c * 512:(c + 1) * 512],
                                w, rhs,
                                start=True, stop=True,
                                tile_position=(b, 0),
                            )
                        sl = slice(t * DR, (t + 1) * DR)
                        nc.scalar.copy(dst[:, sl], pt[:])
                        # per-slab, right behind the drain:
                        if xt > 0:
                            nc.vector.tensor_tensor(
                                colacc[:, sl], colacc[:, sl], dst[:, sl],
                                op=OP.min)
                        if t % 2 == 1:
                            # leaf merge of slabs t-1, t -> fold slab t//2
                            i = t // 2
                            nc.vector.tensor_tensor(
                                fold[:, i * DR:(i + 1) * DR],
                                dst[:, (t - 1) * DR:t * DR],
                                dst[:, t * DR:(t + 1) * DR], op=OP.min)
                    # row-min: internal merges + reduce; mm1 on DVE, the
                    # rest trails on GPSIMD (runs under later x-tiles).
                    nc.vector.tensor_tensor(
                        fold[:, 0:DR], fold[:, 0:DR], fold[:, DR:2 * DR],
                        op=OP.min)
                    if gp_folds:
                        nc.gpsimd.tensor_tensor(
                            fold[:, 2 * DR:3 * DR], fold[:, 2 * DR:3 * DR],
                            fold[:, 3 * DR:4 * DR], op=OP.min)
                        nc.gpsimd.tensor_tensor(
                            fold[:, 0:DR], fold[:, 0:DR],
                            fold[:, 2 * DR:3 * DR], op=OP.min)
                        w2 = DR // 2
                        while w2 >= 256:
                            nc.gpsimd.tensor_tensor(
                                fold[:, :w2], fold[:, :w2], fold[:, w2:2 * w2],
                                op=OP.min)
                            w2 //= 2
                        nc.vector.tensor_reduce(
                            rmin[:, xt:xt + 1], fold[:, :256], axis=AX.X,
                            op=OP.min)
                    else:
                        nc.vector.tensor_tensor(
                            fold[:, 2 * DR:3 * DR], fold[:, 2 * DR:3 * DR],
                            fold[:, 3 * DR:4 * DR], op=OP.min)
                        nc.vector.tensor_tensor(
                            fold[:, 0:DR], fold[:, 0:DR],
                            fold[:, 2 * DR:3 * DR], op=OP.min)
                        w2 = DR // 2
                        while w2 >= 256:
                            nc.vector.tensor_tensor(
                                fold[:, :w2], fold[:, :w2], fold[:, w2:2 * w2],
                                op=OP.min)
                            w2 //= 2
                        nc.vector.tensor_reduce(
                            rmin[:, xt:xt + 1], fold[:, :256], axis=AX.X,
                            op=OP.min)

            # ---------- epilogue: partition-min of colacc via PE transpose ----------
            colmin16 = out_pool.tile([128, yf], fp16, tag="colmin16")
            with tc.tile_pool(name="psumT", bufs=2, space="PSUM") as psumt_pool:
                bb = 16  # transposed blocks per batch
                nb = yf // bb
                for b in range(nb):
                    ptile = psumt_pool.tile([128, bb * 128], fp16, tag="ptile")
                    for q in range(bb):
                        blk = b * bb + q
                        nc.tensor.transpose(
                            ptile[:, q * 128:(q + 1) * 128],
                            colacc[:, blk * 128:(blk + 1) * 128],
                            ident[:],
                        )
                    nc.vector.tensor_reduce(
                        colmin16[:, b * bb:(b + 1) * bb],
                        ptile[:].rearrange("p (a f) -> p a f", a=bb),
                        axis=AX.X, op=OP.min,
                    )

            # ---------- clamp + sqrt + store ----------
            colmin32 = out_pool.tile([128, yf], fp32, tag="colmin32")
            nc.vector.tensor_scalar_max(colmin32[:], colmin16[:], 0.0)
            colout = out_pool.tile([128, yf], fp32, tag="colout")
            nc.scalar.activation(colout[:], colmin32[:], ACT.Sqrt)
            nc.sync.dma_start(col_h.ap()[:, :], colout[:])

            rclamp = out_pool.tile([128, xt_n], fp32, tag="rclamp")
            nc.vector.tensor_scalar_max(rclamp[:], rmin[:], 0.0)
            rowout = out_pool.tile([128, xt_n], fp32, tag="rowout")
            nc.scalar.activation(rowout[:], rclamp[:], ACT.Sqrt)
            nc.sync.dma_start(row_h.ap()[:, :], rowout[:])

    nc.compile()
    return nc


_CACHED = None


def _get_module():
    global _CACHED
    if _CACHED is None:
        _CACHED = build_module()
    return _CACHED


def run_on_hw(nc, in_maps, **kw):
    from concourse.bass_utils import run_bass_kernel_spmd
    return run_bass_kernel_spmd(nc, in_maps, core_ids=list(range(N_CORES)), **kw)


def _postprocess(results):
    rowcat = np.concatenate(
        [results[c]["row_out"].T.reshape(-1) for c in range(N_CORES)])
    colmin = np.stack(
        [results[c]["col_out"].T.reshape(-1) for c in range(N_CORES)]).min(axis=0)
    loss = rowcat.mean(dtype=np.float64) + colmin.mean(dtype=np.float64)
    return np.asarray(loss, dtype=np.float32)


def kernel(pred_corners, gt_corners):
    x = np.ascontiguousarray(np.asarray(pred_corners, dtype=np.float32).reshape(-1, 3))
    y = np.ascontiguousarray(np.asarray(gt_corners, dtype=np.float32).reshape(-1, 3))
    assert x.shape == (NX, 3) and y.shape == (NY, 3)
    nc = _get_module()
    in_maps = [
        {"x_shard": x[c * RP:(c + 1) * RP], "y_full": y} for c in range(N_CORES)
    ]
    res = run_on_hw(nc, in_maps)
    return _postprocess(res.results)


# revision 12
# speedup vs baseline: 1.0436x; 1.0027x over previous
"""Chamfer loss kernel for Trainium2, 8 NeuronCores.

Strategy (sharding_hint): row-block the 16384x16384 distance matrix.
Core c owns x rows [c*2048, (c+1)*2048) (x = flattened pred corners) and
all 16384 y points (flattened gt corners). Each core computes, on device:
  - d2[n, m] = |x_n|^2 + |y_m|^2 - 2 x.y  for its row block, via a K=16
    fp16 matmul using hi/lo fp16 splits of the operands (~fp32 accuracy).
    The PE array is row-tiled 4x (tile_position=(32c,0)): row group c
    handles the 4096-column chunk c, so 4 matmuls stream concurrently in
    the 128x128 array (K=16 uses only 16 of 32 rows per group).
  - per-PSUM-tile drain to SBUF fp16 on the scalar (ACT) engine,
  - row mins via a single fused tensor_tensor_reduce (halves fold + min
    accumulate) per x-tile on the DVE,
  - a running column min (TT-min) per x-tile on the DVE,
  - column partition-min via PE transposes + tensor_reduce epilogue.
Host glue: shard x, gather per-core row mins, all-reduce(min) the partial
column mins across the 8 cores, then mean both and add - the loss.
"""

import sys
import numpy as np

if "/opt/trn_rl_repo" not in sys.path:
    sys.path.insert(0, "/opt/trn_rl_repo")

# ---- hardcoded problem geometry (from the task spec) ----
N_CORES = 8
NX = 16384          # total x points (2048 boxes * 8 corners)
NY = 16384          # total y points
RP = NX // N_CORES  # 2048 x rows per core
XT = RP // 128      # 16 x tiles of 128 rows
K = 16              # contraction rows of the split matmul
NGRP = 4            # PE row groups (4x tiling); group c owns cols [c*4096, +4096)
CHUNK = NY // NGRP  # 4096 cols per row group
DR = 2048           # columns per drain tile (4 PSUM banks)
NDR = NY // DR      # 8 drain steps per x-tile


def build_module(rp=RP, ny=NY, n_grp=NGRP, use_ttr=False, gp_folds=False):
    # NOTE: gp_folds=True (nc.gpsimd.tensor_tensor fp16 min) also crashes on
    # TRN2 hardware via this runner; row-tree stays on the DVE.
    # NOTE: use_ttr=True (InstTensorTensorReduce) crashes on TRN2 hardware
    # (and is 1x-rate anyway); keep the TT-min halving tree.
    """Build + compile the per-core Bass module. Returns the Bacc object.

    n_grp: number of PE row groups (4 = full row tiling, 1 = none).
    use_ttr: row-min via fused tensor_tensor_reduce vs TT-min tree.
    """
    from contextlib import ExitStack

    import concourse.tile as tile
    from concourse import bacc, mybir
    from concourse.masks import make_identity

    fp32 = mybir.dt.float32
    fp16 = mybir.dt.float16
    AX = mybir.AxisListType
    OP = mybir.AluOpType
    ACT = mybir.ActivationFunctionType

    xt_n = rp // 128
    xf = rp // 128       # free cols per partition for x feature tiles
    yf = ny // 128
    chunk = ny // n_grp  # columns per PE row group
    ndr = ny // DR       # drain steps per x-tile

    nc = bacc.Bacc("TRN2", target_bir_lowering=False, debug=False,
                   num_devices=N_CORES)
    x_h = nc.dram_tensor("x_shard", [rp, 3], fp32, kind="ExternalInput")
    y_h = nc.dram_tensor("y_full", [ny, 3], fp32, kind="ExternalInput")
    row_h = nc.dram_tensor("row_out", [128, xt_n], fp32, kind="ExternalOutput")
    col_h = nc.dram_tensor("col_out", [128, yf], fp32, kind="ExternalOutput")

    with tile.TileContext(nc) as tc:
        with ExitStack() as ctx:
            const_pool = ctx.enter_context(tc.tile_pool(name="const", bufs=1))
            prep_pool = ctx.enter_context(tc.tile_pool(name="prep", bufs=1))
            big_pool = ctx.enter_context(tc.tile_pool(name="big", bufs=1))
            dst_pool = ctx.enter_context(tc.tile_pool(name="dst", bufs=2))
            fold_pool = ctx.enter_context(tc.tile_pool(name="fold", bufs=2))
            out_pool = ctx.enter_context(tc.tile_pool(name="outp", bufs=1))

            # ---------- constants ----------
            ones_y = const_pool.tile([128, yf], fp16, tag="ones_y")
            nc.vector.memset(ones_y[:], 1.0)
            ident = const_pool.tile([128, 128], fp16, tag="ident")
            make_identity(nc, ident[:])

            # ---------- feature prep: x ----------
            craw_x = prep_pool.tile([128, 3 * xf], fp32, tag="craw_x")
            nc.sync.dma_start(
                craw_x[:], x_h.ap().rearrange("(p f) d -> p (f d)", p=128))
            cx = prep_pool.tile([128, 3 * xf], fp32, tag="cx")
            craw_x3 = craw_x[:].rearrange("p (f d) -> p d f", d=3)
            for d in range(3):
                nc.vector.tensor_copy(cx[:, d * xf:(d + 1) * xf],
                                      craw_x3[:, d:d + 1, :])
            n2x = prep_pool.tile([128, xf], fp32, tag="n2x")
            tmpx = prep_pool.tile([128, xf], fp32, tag="tmpx")
            nc.vector.tensor_tensor(n2x[:], cx[:, 0:xf], cx[:, 0:xf], op=OP.mult)
            nc.vector.tensor_tensor(tmpx[:], cx[:, xf:2 * xf], cx[:, xf:2 * xf], op=OP.mult)
            nc.vector.tensor_tensor(n2x[:], n2x[:], tmpx[:], op=OP.add)
            nc.vector.tensor_tensor(tmpx[:], cx[:, 2 * xf:3 * xf], cx[:, 2 * xf:3 * xf], op=OP.mult)
            nc.vector.tensor_tensor(n2x[:], n2x[:], tmpx[:], op=OP.add)
            n2xh = prep_pool.tile([128, xf], fp16, tag="n2xh")
            n2xh32 = prep_pool.tile([128, xf], fp32, tag="n2xh32")
            n2xl = prep_pool.tile([128, xf], fp16, tag="n2xl")
            nc.vector.tensor_copy(n2xh[:], n2x[:])
            nc.scalar.copy(n2xh32[:], n2xh[:])
            nc.vector.tensor_tensor(n2xl[:], n2x[:], n2xh32[:], op=OP.subtract)
            # a = -2x, then hi/lo split
            ax = prep_pool.tile([128, 3 * xf], fp32, tag="ax")
            nc.vector.tensor_scalar_mul(ax[:], cx[:], -2.0)
            axh = prep_pool.tile([128, 3 * xf], fp16, tag="axh")
            axh32 = prep_pool.tile([128, 3 * xf], fp32, tag="axh32")
            axl = prep_pool.tile([128, 3 * xf], fp16, tag="axl")
            nc.vector.tensor_copy(axh[:], ax[:])
            nc.scalar.copy(axh32[:], axh[:])
            nc.vector.tensor_tensor(axl[:], ax[:], axh32[:], op=OP.subtract)

            # ---------- feature prep: y ----------
            # cy[p, d*yf + f] = y[p*yf + f, d]
            # one contiguous DMA (fast), then de-interleave xyz on the DVE -
            # the 4B/12B strided DRAM read pattern costs ~14us per plane.
            craw_y = prep_pool.tile([128, 3 * yf], fp32, tag="craw_y")
            nc.sync.dma_start(
                craw_y[:], y_h.ap().rearrange("(p f) d -> p (f d)", p=128))
            cy = prep_pool.tile([128, 3 * yf], fp32, tag="cy")
            craw_y3 = craw_y[:].rearrange("p (f d) -> p d f", d=3)
            for d in range(3):
                nc.vector.tensor_copy(cy[:, d * yf:(d + 1) * yf],
                                      craw_y3[:, d:d + 1, :])
            n2y = prep_pool.tile([128, yf], fp32, tag="n2y")
            tmpy = prep_pool.tile([128, yf], fp32, tag="tmpy")
            nc.vector.tensor_tensor(n2y[:], cy[:, 0:yf], cy[:, 0:yf], op=OP.mult)
            nc.vector.tensor_tensor(tmpy[:], cy[:, yf:2 * yf], cy[:, yf:2 * yf], op=OP.mult)
            nc.vector.tensor_tensor(n2y[:], n2y[:], tmpy[:], op=OP.add)
            nc.vector.tensor_tensor(tmpy[:], cy[:, 2 * yf:3 * yf], cy[:, 2 * yf:3 * yf], op=OP.mult)
            nc.vector.tensor_tensor(n2y[:], n2y[:], tmpy[:], op=OP.add)
            # hi/lo split of n2y
            n2yh = prep_pool.tile([128, yf], fp16, tag="n2yh")
            n2yh32 = prep_pool.tile([128, yf], fp32, tag="n2yh32")
            n2yl = prep_pool.tile([128, yf], fp16, tag="n2yl")
            nc.vector.tensor_copy(n2yh[:], n2y[:])
            nc.scalar.copy(n2yh32[:], n2yh[:])
            nc.vector.tensor_tensor(n2yl[:], n2y[:], n2yh32[:], op=OP.subtract)
            # hi/lo split of y coords (all 3 at once)
            yh = prep_pool.tile([128, 3 * yf], fp16, tag="yh")
            yh32 = prep_pool.tile([128, 3 * yf], fp32, tag="yh32")
            yl = prep_pool.tile([128, 3 * yf], fp16, tag="yl")
            nc.vector.tensor_copy(yh[:], cy[:])
            nc.scalar.copy(yh32[:], yh[:])
            nc.vector.tensor_tensor(yl[:], cy[:], yh32[:], op=OP.subtract)

            # ---------- assemble K x N operand tiles ----------
            # pairing per K row r:  phi[r] . psi[r]
            #  r0 : 1      * |y|2_h     r1 : 1      * |y|2_l
            #  r2 : |x|2_h * 1          r3 : |x|2_l * 1
            #  r4..6  : axh_d * yh_d    r7..9  : axh_d * yl_d
            #  r10..12: axl_d * yh_d    r13..15: axl_d * yl_d
            # Round-trip through DRAM scratch: the SBUF->DRAM writes keep the
            # [128, f] layout (768B/partition descriptors), and each psi/phi
            # row read becomes one small 2D strided DRAM read - far cheaper
            # than a [128-partition gather] -> [1 partition] SBUF-SBUF DMA.
            # For the 4x PE row tiling, row k of PE group c lives on SBUF
            # partition 32c+k: phi rows are replicated to all 4 bases, psi
            # rows are split by column chunk (group c gets chunk c).
            dram_pool = ctx.enter_context(
                tc.tile_pool(name="dscr", bufs=1, space="DRAM"))
            d_yh = dram_pool.tile([128, 3 * yf], fp16, tag="d_yh")
            d_yl = dram_pool.tile([128, 3 * yf], fp16, tag="d_yl")
            d_n2yh = dram_pool.tile([128, yf], fp16, tag="d_n2yh")
            d_n2yl = dram_pool.tile([128, yf], fp16, tag="d_n2yl")
            d_ones = dram_pool.tile([128, yf], fp16, tag="d_ones")
            d_xh = dram_pool.tile([128, 3 * xf], fp16, tag="d_xh")
            d_xl = dram_pool.tile([128, 3 * xf], fp16, tag="d_xl")
            d_n2xh = dram_pool.tile([128, xf], fp16, tag="d_n2xh")
            d_n2xl = dram_pool.tile([128, xf], fp16, tag="d_n2xl")
            nc.sync.dma_start(d_ones[:], ones_y[:])
            nc.sync.dma_start(d_xh[:], axh[:])
            nc.sync.dma_start(d_xl[:], axl[:])
            nc.sync.dma_start(d_n2xh[:], n2xh[:])
            nc.sync.dma_start(d_n2xl[:], n2xl[:])
            nc.gpsimd.dma_start(d_yh[:], yh[:])
            nc.gpsimd.dma_start(d_yl[:], yl[:])
            nc.scalar.dma_start(d_n2yh[:], n2yh[:])
            nc.scalar.dma_start(d_n2yl[:], n2yl[:])

            # phi_rep[32c + k, :] = phi row k (same for all c)
            # psi_rep[32c + k, :] = psi row k, global columns [c*4096, +4096)
            # Assemble both at base 0 first (one strided-DRAM read per row),
            # then replicate/shift to bases 32/64/96 with a few fat
            # SBUF->SBUF DMAs (16 partitions x contiguous bytes each).
            phi_rep = big_pool.tile([128, rp], fp16, tag="phi_rep")
            psi_mst = big_pool.tile([128, ny], fp16, tag="psi_mst")
            psi_rep = big_pool.tile([128, chunk], fp16, tag="psi_rep")

            _eng = [nc.sync, nc.gpsimd, nc.scalar]
            _rr = [0]

            def dma(dst, src):
                e = _eng[_rr[0] % len(_eng)]
                _rr[0] += 1
                e.dma_start(dst, src)

            def flat(t):       # [128, yf] dram tile -> full linear row
                return t[:, :].rearrange("p f -> (p f)")

            def plane3(t, d):  # [128, 3f] d-major dram tile -> full coord row
                return t[:].rearrange("p (d f) -> d p f", d=3)[d:d + 1, :, :]

            ones_rp = d_ones[0:rp // yf, :].rearrange("p f -> (p f)")

            def ph(k):
                return phi_rep[k:k + 1, :]

            def ps(k):
                return psi_mst[k:k + 1, :]

            dma(ph(0), ones_rp)
            dma(ph(1), ones_rp)
            dma(ph(2), d_n2xh[:, :].rearrange("p f -> (p f)"))
            dma(ph(3), d_n2xl[:, :].rearrange("p f -> (p f)"))
            for d in range(3):
                dma(ph(4 + d), plane3(d_xh, d))
                dma(ph(7 + d), plane3(d_xh, d))
                dma(ph(10 + d), plane3(d_xl, d))
                dma(ph(13 + d), plane3(d_xl, d))

            dma(ps(0), flat(d_n2yh))
            dma(ps(1), flat(d_n2yl))
            dma(ps(2), flat(d_ones))
            dma(ps(3), flat(d_ones))
            for d in range(3):
                dma(ps(4 + d), plane3(d_yh, d))
                dma(ps(7 + d), plane3(d_yl, d))
                dma(ps(10 + d), plane3(d_yh, d))
                dma(ps(13 + d), plane3(d_yl, d))

            # replicate phi to bases 32/64/96; shift psi chunks c>=1 there.
            # group 0 reads psi_mst / phi_rep base 0 directly.
            for c in range(1, n_grp):
                nc.sync.dma_start(phi_rep[32 * c:32 * c + K, :],
                                  phi_rep[0:K, :])
                nc.gpsimd.dma_start(
                    psi_rep[32 * c:32 * c + K, :],
                    psi_mst[0:K, c * chunk:(c + 1) * chunk])

            # ---------- main loop ----------
            # Per x-tile: 8 drain steps r; step r is PE row group c=r%4,
            # chunk-half h=r//4, i.e. dst cols [r*2048, +2048) = global cols
            # [c*4096 + h*2048, +2048)  (a fixed column permutation - min and
            # mean are permutation invariant, and all cores use the same one).
            colacc = big_pool.tile([128, ny], fp16, tag="colacc")
            rmin = out_pool.tile([128, xt_n], fp32, tag="rmin")

            with tc.tile_pool(name="psum", bufs=2, space="PSUM") as psum_pool:
                for xt in range(xt_n):
                    dst = colacc if xt == 0 else dst_pool.tile(
                        [128, ny], fp16, tag="dst")
                    fold = fold_pool.tile([128, ny // 2], fp16, tag="fold")
                    for t in range(ndr):
                        # psum tile t: column c*512+j of the tile holds
                        # global column c*4096 + t*512 + j -> the 4 matmuls
                        # hit 4 different PE row groups and run concurrently.
                        pt = psum_pool.tile([128, DR], fp32, tag="pt")
                        for c in range(n_grp):
                            b = 32 * c
                            w = phi_rep[b:b + K, xt * 128:(xt + 1) * 128]
                            src_ps = psi_mst if c == 0 else psi_rep
                            rhs = src_ps[b:b + K, t * 512:(t + 1) * 512]
                            nc.tensor.matmul(
                                pt[:, c * 512:(c + 1) * 512],
                                w, rhs,
                                start=True, stop=True,
                                tile_position=(b, 0),
                            )
                        sl = slice(t * DR, (t + 1) * DR)
                        nc.scalar.copy(dst[:, sl], pt[:])
                    # col-min accumulate (full width; chunked on the last
                    # x-tile so the epilogue can start under it)
                    if xt == xt_n - 1:
                        for cch in range(4):
                            sl = slice(cch * ny // 4, (cch + 1) * ny // 4)
                            nc.vector.tensor_tensor(
                                colacc[:, sl], colacc[:, sl], dst[:, sl],
                                op=OP.min)
                    elif xt > 0:
                        nc.vector.tensor_tensor(
                            colacc[:], colacc[:], dst[:], op=OP.min)
                    # row-min: TT-min halving tree (fold1..4) + 1x reduce.
                    nc.vector.tensor_tensor(
                        fold[:], dst[:, :ny // 2], dst[:, ny // 2:],
                        op=OP.min)
                    w2 = ny // 4
                    while w2 >= 1024:
                        nc.vector.tensor_tensor(
                            fold[:, :w2], fold[:, :w2], fold[:, w2:2 * w2],
                            op=OP.min)
                        w2 //= 2
                    nc.vector.tensor_reduce(
                        rmin[:, xt:xt + 1], fold[:, :1024], axis=AX.X,
                        op=OP.min)

            # ---------- epilogue: partition-min of colacc via PE transpose ----------
            colmin16 = out_pool.tile([128, yf], fp16, tag="colmin16")
            with tc.tile_pool(name="psumT", bufs=2, space="PSUM") as psumt_pool:
                bb = 16  # transposed blocks per batch
                nb = yf // bb
                for b in range(nb):
                    ptile = psumt_pool.tile([128, bb * 128], fp16, tag="ptile")
                    for q in range(bb):
                        blk = b * bb + q
                        nc.tensor.transpose(
                            ptile[:, q * 128:(q + 1) * 128],
                            colacc[:, blk * 128:(blk + 1) * 128],
                            ident[:],
                        )
                    nc.vector.tensor_reduce(
                        colmin16[:, b * bb:(b + 1) * bb],
                        ptile[:].rearrange("p (a f) -> p a f", a=bb),
                        axis=AX.X, op=OP.min,
                    )

            # ---------- clamp + sqrt + store ----------
            colmin32 = out_pool.tile([128, yf], fp32, tag="colmin32")
            nc.vector.tensor_scalar_max(colmin32[:], colmin16[:], 0.0)
            colout = out_pool.tile([128, yf], fp32, tag="colout")
            nc.scalar.activation(colout[:], colmin32[:], ACT.Sqrt)
            nc.sync.dma_start(col_h.ap()[:, :], colout[:])

            rclamp = out_pool.tile([128, xt_n], fp32, tag="rclamp")
            nc.vector.tensor_scalar_max(rclamp[:], rmin[:], 0.0)
            rowout = out_pool.tile([128, xt_n], fp32, tag="rowout")
            nc.scalar.activation(rowout[:], rclamp[:], ACT.Sqrt)
            nc.sync.dma_start(row_h.ap()[:, :], rowout[:])

    nc.compile()
    return nc


_CACHED = None


def _get_module():
    global _CACHED
    if _CACHED is None:
        _CACHED = build_module()
    return _CACHED


def run_on_hw(nc, in_maps, **kw):
    from concourse.bass_utils import run_bass_kernel_spmd
    return run_bass_kernel_spmd(nc, in_maps, core_ids=list(range(N_CORES)), **kw)


def _postprocess(results):
    rowcat = np.concatenate(
        [results[c]["row_out"].T.reshape(-1) for c in range(N_CORES)])
    colmin = np.stack(
        [results[c]["col_out"].T.reshape(-1) for c in range(N_CORES)]).min(axis=0)
    loss = rowcat.mean(dtype=np.float64) + colmin.mean(dtype=np.float64)
    return np.asarray(loss, dtype=np.float32)


def kernel(pred_corners, gt_corners):
    x = np.ascontiguousarray(np.asarray(pred_corners, dtype=np.float32).reshape(-1, 3))
    y = np.ascontiguousarray(np.asarray(gt_corners, dtype=np.float32).reshape(-1, 3))
    assert x.shape == (NX, 3) and y.shape == (NY, 3)
    nc = _get_module()
    in_maps = [
        {"x_shard": x[c * RP:(c + 1) * RP], "y_full": y} for c in range(N_CORES)
    ]
    res = run_on_hw(nc, in_maps)
    return _postprocess(res.results)


# revision 16
# speedup vs baseline: 1.0582x; 1.0140x over previous
"""Chamfer loss kernel for Trainium2, 8 NeuronCores.

Strategy (sharding_hint): row-block the 16384x16384 distance matrix.
Core c owns x rows [c*2048, (c+1)*2048) (x = flattened pred corners) and
all 16384 y points (flattened gt corners). Each core computes, on device:
  - d2[n, m] = |x_n|^2 + |y_m|^2 - 2 x.y  for its row block, via a K=16
    fp16 matmul using hi/lo fp16 splits of the operands (~fp32 accuracy).
    The PE array is row-tiled 4x (tile_position=(32c,0)): row group c
    handles the 4096-column chunk c, so 4 matmuls stream concurrently in
    the 128x128 array (K=16 uses only 16 of 32 rows per group).
  - per-PSUM-tile drain to SBUF fp16 on the scalar (ACT) engine,
  - row mins via a single fused tensor_tensor_reduce (halves fold + min
    accumulate) per x-tile on the DVE,
  - a running column min (TT-min) per x-tile on the DVE,
  - column partition-min via PE transposes + tensor_reduce epilogue.
Host glue: shard x, gather per-core row mins, all-reduce(min) the partial
column mins across the 8 cores, then mean both and add - the loss.
"""

import sys
import numpy as np

if "/opt/trn_rl_repo" not in sys.path:
    sys.path.insert(0, "/opt/trn_rl_repo")

# ---- hardcoded problem geometry (from the task spec) ----
N_CORES = 8
NX = 16384          # total x points (2048 boxes * 8 corners)
NY = 16384          # total y points
RP = NX // N_CORES  # 2048 x rows per core
XT = RP // 128      # 16 x tiles of 128 rows
K = 16              # contraction rows of the split matmul
NGRP = 4            # PE row groups (4x tiling); group c owns cols [c*4096, +4096)
CHUNK = NY // NGRP  # 4096 cols per row group
DR = 2048           # columns per drain tile (4 PSUM banks)
NDR = NY // DR      # 8 drain steps per x-tile


def build_module(rp=RP, ny=NY, n_grp=NGRP, use_ttr=False, gp_folds=False):
    # NOTE: gp_folds=True (nc.gpsimd.tensor_tensor fp16 min) also crashes on
    # TRN2 hardware via this runner; row-tree stays on the DVE.
    # NOTE: use_ttr=True (InstTensorTensorReduce) crashes on TRN2 hardware
    # (and is 1x-rate anyway); keep the TT-min halving tree.
    """Build + compile the per-core Bass module. Returns the Bacc object.

    n_grp: number of PE row groups (4 = full row tiling, 1 = none).
    use_ttr: row-min via fused tensor_tensor_reduce vs TT-min tree.
    """
    from contextlib import ExitStack

    import concourse.tile as tile
    from concourse import bacc, mybir
    from concourse.masks import make_identity

    fp32 = mybir.dt.float32
    fp16 = mybir.dt.float16
    AX = mybir.AxisListType
    OP = mybir.AluOpType
    ACT = mybir.ActivationFunctionType

    xt_n = rp // 128
    xf = rp // 128       # free cols per partition for x feature tiles
    yf = ny // 128
    chunk = ny // n_grp  # columns per PE row group
    ndr = ny // DR       # drain steps per x-tile

    nc = bacc.Bacc("TRN2", target_bir_lowering=False, debug=False,
                   num_devices=N_CORES)
    x_h = nc.dram_tensor("x_shard", [rp, 3], fp32, kind="ExternalInput")
    y_h = nc.dram_tensor("y_full", [ny, 3], fp32, kind="ExternalInput")
    row_h = nc.dram_tensor("row_out", [128, xt_n], fp32, kind="ExternalOutput")
    col_h = nc.dram_tensor("col_out", [128, yf], fp32, kind="ExternalOutput")

    with tile.TileContext(nc) as tc:
        with ExitStack() as ctx:
            const_pool = ctx.enter_context(tc.tile_pool(name="const", bufs=1))
            prep_pool = ctx.enter_context(tc.tile_pool(name="prep", bufs=1))
            big_pool = ctx.enter_context(tc.tile_pool(name="big", bufs=1))
            dst_pool = ctx.enter_context(tc.tile_pool(name="dst", bufs=2))
            fold_pool = ctx.enter_context(tc.tile_pool(name="fold", bufs=2))
            out_pool = ctx.enter_context(tc.tile_pool(name="outp", bufs=1))

            # ---------- constants ----------
            ones_y = const_pool.tile([128, yf], fp16, tag="ones_y")
            nc.vector.memset(ones_y[:], 1.0)
            ident = const_pool.tile([128, 128], fp16, tag="ident")
            make_identity(nc, ident[:])

            # ---------- feature prep: x ----------
            craw_x = prep_pool.tile([128, 3 * xf], fp32, tag="craw_x")
            nc.sync.dma_start(
                craw_x[:], x_h.ap().rearrange("(p f) d -> p (f d)", p=128))
            cx = prep_pool.tile([128, 3 * xf], fp32, tag="cx")
            craw_x3 = craw_x[:].rearrange("p (f d) -> p d f", d=3)
            for d in range(3):
                nc.vector.tensor_copy(cx[:, d * xf:(d + 1) * xf],
                                      craw_x3[:, d:d + 1, :])
            n2x = prep_pool.tile([128, xf], fp32, tag="n2x")
            tmpx = prep_pool.tile([128, xf], fp32, tag="tmpx")
            nc.vector.tensor_tensor(n2x[:], cx[:, 0:xf], cx[:, 0:xf], op=OP.mult)
            nc.vector.tensor_tensor(tmpx[:], cx[:, xf:2 * xf], cx[:, xf:2 * xf], op=OP.mult)
            nc.vector.tensor_tensor(n2x[:], n2x[:], tmpx[:], op=OP.add)
            nc.vector.tensor_tensor(tmpx[:], cx[:, 2 * xf:3 * xf], cx[:, 2 * xf:3 * xf], op=OP.mult)
            nc.vector.tensor_tensor(n2x[:], n2x[:], tmpx[:], op=OP.add)
            n2xh = prep_pool.tile([128, xf], fp16, tag="n2xh")
            n2xh32 = prep_pool.tile([128, xf], fp32, tag="n2xh32")
            n2xl = prep_pool.tile([128, xf], fp16, tag="n2xl")
            nc.vector.tensor_copy(n2xh[:], n2x[:])
            nc.scalar.copy(n2xh32[:], n2xh[:])
            nc.vector.tensor_tensor(n2xl[:], n2x[:], n2xh32[:], op=OP.subtract)
            # a = -2x, then hi/lo split
            ax = prep_pool.tile([128, 3 * xf], fp32, tag="ax")
            nc.vector.tensor_scalar_mul(ax[:], cx[:], -2.0)
            axh = prep_pool.tile([128, 3 * xf], fp16, tag="axh")
            axh32 = prep_pool.tile([128, 3 * xf], fp32, tag="axh32")
            axl = prep_pool.tile([128, 3 * xf], fp16, tag="axl")
            nc.vector.tensor_copy(axh[:], ax[:])
            nc.scalar.copy(axh32[:], axh[:])
            nc.vector.tensor_tensor(axl[:], ax[:], axh32[:], op=OP.subtract)

            # ---------- feature prep: y ----------
            # cy[p, d*yf + f] = y[p*yf + f, d]
            # one contiguous DMA (fast), then de-interleave xyz on the DVE -
            # the 4B/12B strided DRAM read pattern costs ~14us per plane.
            craw_y = prep_pool.tile([128, 3 * yf], fp32, tag="craw_y")
            nc.sync.dma_start(
                craw_y[:], y_h.ap().rearrange("(p f) d -> p (f d)", p=128))
            cy = prep_pool.tile([128, 3 * yf], fp32, tag="cy")
            craw_y3 = craw_y[:].rearrange("p (f d) -> p d f", d=3)
            for d in range(3):
                nc.vector.tensor_copy(cy[:, d * yf:(d + 1) * yf],
                                      craw_y3[:, d:d + 1, :])
            n2y = prep_pool.tile([128, yf], fp32, tag="n2y")
            tmpy = prep_pool.tile([128, yf], fp32, tag="tmpy")
            nc.vector.tensor_tensor(n2y[:], cy[:, 0:yf], cy[:, 0:yf], op=OP.mult)
            nc.vector.tensor_tensor(tmpy[:], cy[:, yf:2 * yf], cy[:, yf:2 * yf], op=OP.mult)
            nc.vector.tensor_tensor(n2y[:], n2y[:], tmpy[:], op=OP.add)
            nc.vector.tensor_tensor(tmpy[:], cy[:, 2 * yf:3 * yf], cy[:, 2 * yf:3 * yf], op=OP.mult)
            nc.vector.tensor_tensor(n2y[:], n2y[:], tmpy[:], op=OP.add)
            # hi/lo split of n2y
            n2yh = prep_pool.tile([128, yf], fp16, tag="n2yh")
            n2yh32 = prep_pool.tile([128, yf], fp32, tag="n2yh32")
            n2yl = prep_pool.tile([128, yf], fp16, tag="n2yl")
            nc.vector.tensor_copy(n2yh[:], n2y[:])
            nc.scalar.copy(n2yh32[:], n2yh[:])
            nc.vector.tensor_tensor(n2yl[:], n2y[:], n2yh32[:], op=OP.subtract)
            # hi/lo split of y coords (all 3 at once)
            yh = prep_pool.tile([128, 3 * yf], fp16, tag="yh")
            yh32 = prep_pool.tile([128, 3 * yf], fp32, tag="yh32")
            yl = prep_pool.tile([128, 3 * yf], fp16, tag="yl")
            nc.vector.tensor_copy(yh[:], cy[:])
            nc.scalar.copy(yh32[:], yh[:])
            nc.vector.tensor_tensor(yl[:], cy[:], yh32[:], op=OP.subtract)

            # ---------- assemble K x N operand tiles ----------
            # pairing per K row r:  phi[r] . psi[r]
            #  r0 : 1      * |y|2_h     r1 : 1      * |y|2_l
            #  r2 : |x|2_h * 1          r3 : |x|2_l * 1
            #  r4..6  : axh_d * yh_d    r7..9  : axh_d * yl_d
            #  r10..12: axl_d * yh_d    r13..15: axl_d * yl_d
            # Round-trip through DRAM scratch: the SBUF->DRAM writes keep the
            # [128, f] layout (768B/partition descriptors), and each psi/phi
            # row read becomes one small 2D strided DRAM read - far cheaper
            # than a [128-partition gather] -> [1 partition] SBUF-SBUF DMA.
            # For the 4x PE row tiling, row k of PE group c lives on SBUF
            # partition 32c+k: phi rows are replicated to all 4 bases, psi
            # rows are split by column chunk (group c gets chunk c).
            dram_pool = ctx.enter_context(
                tc.tile_pool(name="dscr", bufs=1, space="DRAM"))
            d_yh = dram_pool.tile([128, 3 * yf], fp16, tag="d_yh")
            d_yl = dram_pool.tile([128, 3 * yf], fp16, tag="d_yl")
            d_n2yh = dram_pool.tile([128, yf], fp16, tag="d_n2yh")
            d_n2yl = dram_pool.tile([128, yf], fp16, tag="d_n2yl")
            d_ones = dram_pool.tile([128, yf], fp16, tag="d_ones")
            d_xh = dram_pool.tile([128, 3 * xf], fp16, tag="d_xh")
            d_xl = dram_pool.tile([128, 3 * xf], fp16, tag="d_xl")
            d_n2xh = dram_pool.tile([128, xf], fp16, tag="d_n2xh")
            d_n2xl = dram_pool.tile([128, xf], fp16, tag="d_n2xl")
            nc.sync.dma_start(d_ones[:], ones_y[:])
            nc.sync.dma_start(d_xh[:], axh[:])
            nc.sync.dma_start(d_xl[:], axl[:])
            nc.sync.dma_start(d_n2xh[:], n2xh[:])
            nc.sync.dma_start(d_n2xl[:], n2xl[:])
            nc.gpsimd.dma_start(d_yh[:], yh[:])
            nc.gpsimd.dma_start(d_yl[:], yl[:])
            nc.scalar.dma_start(d_n2yh[:], n2yh[:])
            nc.scalar.dma_start(d_n2yl[:], n2yl[:])

            # phi_rep[32c + k, :] = phi row k (same for all c)
            # psi_rep[32c + k, :] = psi row k, global columns [c*4096, +4096)
            # Assemble both at base 0 first (one strided-DRAM read per row),
            # then replicate/shift to bases 32/64/96 with a few fat
            # SBUF->SBUF DMAs (16 partitions x contiguous bytes each).
            phi_rep = big_pool.tile([128, rp], fp16, tag="phi_rep")
            psi_mst = big_pool.tile([128, ny], fp16, tag="psi_mst")
            psi_rep = big_pool.tile([128, chunk], fp16, tag="psi_rep")

            _eng = [nc.sync, nc.gpsimd, nc.scalar]
            _rr = [0]

            def dma(dst, src):
                e = _eng[_rr[0] % len(_eng)]
                _rr[0] += 1
                e.dma_start(dst, src)

            def flat(t):       # [128, yf] dram tile -> full linear row
                return t[:, :].rearrange("p f -> (p f)")

            def plane3(t, d):  # [128, 3f] d-major dram tile -> full coord row
                return t[:].rearrange("p (d f) -> d p f", d=3)[d:d + 1, :, :]

            ones_rp = d_ones[0:rp // yf, :].rearrange("p f -> (p f)")

            def ph(k):
                return phi_rep[k:k + 1, :]

            def ps(k):
                return psi_mst[k:k + 1, :]

            dma(ph(0), ones_rp)
            dma(ph(1), ones_rp)
            dma(ph(2), d_n2xh[:, :].rearrange("p f -> (p f)"))
            dma(ph(3), d_n2xl[:, :].rearrange("p f -> (p f)"))
            for d in range(3):
                dma(ph(4 + d), plane3(d_xh, d))
                dma(ph(7 + d), plane3(d_xh, d))
                dma(ph(10 + d), plane3(d_xl, d))
                dma(ph(13 + d), plane3(d_xl, d))

            dma(ps(0), flat(d_n2yh))
            dma(ps(1), flat(d_n2yl))
            dma(ps(2), flat(d_ones))
            dma(ps(3), flat(d_ones))
            for d in range(3):
                dma(ps(4 + d), plane3(d_yh, d))
                dma(ps(7 + d), plane3(d_yl, d))
                dma(ps(10 + d), plane3(d_yh, d))
                dma(ps(13 + d), plane3(d_yl, d))

            # replicate phi to bases 32/64/96; shift psi chunks c>=1 there.
            # group 0 reads psi_mst / phi_rep base 0 directly.
            for c in range(1, n_grp):
                nc.sync.dma_start(phi_rep[32 * c:32 * c + K, :],
                                  phi_rep[0:K, :])
                nc.gpsimd.dma_start(
                    psi_rep[32 * c:32 * c + K, :],
                    psi_mst[0:K, c * chunk:(c + 1) * chunk])

            # ---------- main loop ----------
            # Per x-tile: 8 drain steps r; step r is PE row group c=r%4,
            # chunk-half h=r//4, i.e. dst cols [r*2048, +2048) = global cols
            # [c*4096 + h*2048, +2048)  (a fixed column permutation - min and
            # mean are permutation invariant, and all cores use the same one).
            colacc = big_pool.tile([128, ny], fp16, tag="colacc")
            rmin = out_pool.tile([128, xt_n], fp32, tag="rmin")

            with tc.tile_pool(name="psum", bufs=2, space="PSUM") as psum_pool:
                for xt in range(xt_n):
                    dst = colacc if xt == 0 else dst_pool.tile(
                        [128, ny], fp16, tag="dst")
                    fold = fold_pool.tile([128, ny // 2], fp16, tag="fold")
                    for t in range(ndr):
                        # psum tile t: column c*512+j of the tile holds
                        # global column c*4096 + t*512 + j -> the 4 matmuls
                        # hit 4 different PE row groups and run concurrently.
                        pt = psum_pool.tile([128, DR], fp32, tag="pt")
                        for c in range(n_grp):
                            b = 32 * c
                            w = phi_rep[b:b + K, xt * 128:(xt + 1) * 128]
                            src_ps = psi_mst if c == 0 else psi_rep
                            rhs = src_ps[b:b + K, t * 512:(t + 1) * 512]
                            nc.tensor.matmul(
                                pt[:, c * 512:(c + 1) * 512],
                                w, rhs,
                                start=True, stop=True,
                                tile_position=(b, 0),
                            )
                        sl = slice(t * DR, (t + 1) * DR)
                        nc.scalar.copy(dst[:, sl], pt[:])
                    # col-min accumulate (full width; chunked on the last
                    # x-tile so the epilogue can start under it)
                    if xt == xt_n - 1:
                        for cch in range(4):
                            sl = slice(cch * ny // 4, (cch + 1) * ny // 4)
                            nc.vector.tensor_tensor(
                                colacc[:, sl], colacc[:, sl], dst[:, sl],
                                op=OP.min)
                    elif xt > 0:
                        nc.vector.tensor_tensor(
                            colacc[:], colacc[:], dst[:], op=OP.min)
                    # row-min: TT-min halving tree (fold1..4) + 1x reduce.
                    nc.vector.tensor_tensor(
                        fold[:], dst[:, :ny // 2], dst[:, ny // 2:],
                        op=OP.min)
                    w2 = ny // 4
                    while w2 >= 256:
                        nc.vector.tensor_tensor(
                            fold[:, :w2], fold[:, :w2], fold[:, w2:2 * w2],
                            op=OP.min)
                        w2 //= 2
                    nc.vector.tensor_reduce(
                        rmin[:, xt:xt + 1], fold[:, :256], axis=AX.X,
                        op=OP.min)

            # ---------- epilogue: partition-min of colacc via PE transpose ----------
            colmin16 = out_pool.tile([128, yf], fp16, tag="colmin16")
            with tc.tile_pool(name="psumT", bufs=2, space="PSUM") as psumt_pool:
                bb = 16  # transposed blocks per batch
                nb = yf // bb
                for b in range(nb):
                    ptile = psumt_pool.tile([128, bb * 128], fp16, tag="ptile")
                    for q in range(bb):
                        blk = b * bb + q
                        nc.tensor.transpose(
                            ptile[:, q * 128:(q + 1) * 128],
                            colacc[:, blk * 128:(blk + 1) * 128],
                            ident[:],
                        )
                    nc.vector.tensor_reduce(
                        colmin16[:, b * bb:(b + 1) * bb],
                        ptile[:].rearrange("p (a f) -> p a f", a=bb),
                        axis=AX.X, op=OP.min,
                    )

            # ---------- clamp + sqrt + store ----------
            colmin32 = out_pool.tile([128, yf], fp32, tag="colmin32")
            nc.vector.tensor_scalar_max(colmin32[:], colmin16[:], 0.0)
            colout = out_pool.tile([128, yf], fp32, tag="colout")
            nc.scalar.activation(colout[:], colmin32[:], ACT.Sqrt)
            nc.sync.dma_start(col_h.ap()[:, :], colout[:])

            rclamp = out_pool.tile([128, xt_n], fp32, tag="rclamp")
            nc.vector.tensor_scalar_max(rclamp[:], rmin[:], 0.0)
            rowout = out_pool.tile([128, xt_n], fp32, tag="rowout")
            nc.scalar.activation(rowout[:], rclamp[:], ACT.Sqrt)
            nc.sync.dma_start(row_h.ap()[:, :], rowout[:])

    nc.compile()
    return nc


_CACHED = None


def _get_module():
    global _CACHED
    if _CACHED is None:
        _CACHED = build_module()
    return _CACHED


def run_on_hw(nc, in_maps, **kw):
    from concourse.bass_utils import run_bass_kernel_spmd
    return run_bass_kernel_spmd(nc, in_maps, core_ids=list(range(N_CORES)), **kw)


def _postprocess(results):
    rowcat = np.concatenate(
        [results[c]["row_out"].T.reshape(-1) for c in range(N_CORES)])
    colmin = np.stack(
        [results[c]["col_out"].T.reshape(-1) for c in range(N_CORES)]).min(axis=0)
    loss = rowcat.mean(dtype=np.float64) + colmin.mean(dtype=np.float64)
    return np.asarray(loss, dtype=np.float32)


def kernel(pred_corners, gt_corners):
    x = np.ascontiguousarray(np.asarray(pred_corners, dtype=np.float32).reshape(-1, 3))
    y = np.ascontiguousarray(np.asarray(gt_corners, dtype=np.float32).reshape(-1, 3))
    assert x.shape == (NX, 3) and y.shape == (NY, 3)
    nc = _get_module()
    in_maps = [
        {"x_shard": x[c * RP:(c + 1) * RP], "y_full": y} for c in range(N_CORES)
    ]
    res = run_on_hw(nc, in_maps)
    return _postprocess(res.results)


# revision 17
# speedup vs baseline: 1.0587x; 1.0004x over previous
"""Chamfer loss kernel for Trainium2, 8 NeuronCores.

Strategy (sharding_hint): row-block the 16384x16384 distance matrix.
Core c owns x rows [c*2048, (c+1)*2048) (x = flattened pred corners) and
all 16384 y points (flattened gt corners). Each core computes, on device:
  - d2[n, m] = |x_n|^2 + |y_m|^2 - 2 x.y  for its row block, via a K=16
    fp16 matmul using hi/lo fp16 splits of the operands (~fp32 accuracy).
    The PE array is row-tiled 4x (tile_position=(32c,0)): row group c
    handles the 4096-column chunk c, so 4 matmuls stream concurrently in
    the 128x128 array (K=16 uses only 16 of 32 rows per group).
  - per-PSUM-tile drain to SBUF fp16 on the scalar (ACT) engine,
  - row mins via a single fused tensor_tensor_reduce (halves fold + min
    accumulate) per x-tile on the DVE,
  - a running column min (TT-min) per x-tile on the DVE,
  - column partition-min via PE transposes + tensor_reduce epilogue.
Host glue: shard x, gather per-core row mins, all-reduce(min) the partial
column mins across the 8 cores, then mean both and add - the loss.
"""

import sys
import numpy as np

if "/opt/trn_rl_repo" not in sys.path:
    sys.path.insert(0, "/opt/trn_rl_repo")

# ---- hardcoded problem geometry (from the task spec) ----
N_CORES = 8
NX = 16384          # total x points (2048 boxes * 8 corners)
NY = 16384          # total y points
RP = NX // N_CORES  # 2048 x rows per core
XT = RP // 128      # 16 x tiles of 128 rows
K = 16              # contraction rows of the split matmul
NGRP = 4            # PE row groups (4x tiling); group c owns cols [c*4096, +4096)
CHUNK = NY // NGRP  # 4096 cols per row group
DR = 2048           # columns per drain tile (4 PSUM banks)
NDR = NY // DR      # 8 drain steps per x-tile


def build_module(rp=RP, ny=NY, n_grp=NGRP, use_ttr=False, gp_folds=False):
    # NOTE: gp_folds=True (nc.gpsimd.tensor_tensor fp16 min) also crashes on
    # TRN2 hardware via this runner; row-tree stays on the DVE.
    # NOTE: use_ttr=True (InstTensorTensorReduce) crashes on TRN2 hardware
    # (and is 1x-rate anyway); keep the TT-min halving tree.
    """Build + compile the per-core Bass module. Returns the Bacc object.

    n_grp: number of PE row groups (4 = full row tiling, 1 = none).
    use_ttr: row-min via fused tensor_tensor_reduce vs TT-min tree.
    """
    from contextlib import ExitStack

    import concourse.tile as tile
    from concourse import bacc, mybir
    from concourse.masks import make_identity

    fp32 = mybir.dt.float32
    fp16 = mybir.dt.float16
    AX = mybir.AxisListType
    OP = mybir.AluOpType
    ACT = mybir.ActivationFunctionType

    xt_n = rp // 128
    xf = rp // 128       # free cols per partition for x feature tiles
    yf = ny // 128
    chunk = ny // n_grp  # columns per PE row group
    ndr = ny // DR       # drain steps per x-tile

    nc = bacc.Bacc("TRN2", target_bir_lowering=False, debug=False,
                   num_devices=N_CORES)
    x_h = nc.dram_tensor("x_shard", [rp, 3], fp32, kind="ExternalInput")
    y_h = nc.dram_tensor("y_full", [ny, 3], fp32, kind="ExternalInput")
    row_h = nc.dram_tensor("row_out", [128, xt_n], fp32, kind="ExternalOutput")
    col_h = nc.dram_tensor("col_out", [128, yf], fp32, kind="ExternalOutput")

    with tile.TileContext(nc) as tc:
        with ExitStack() as ctx:
            const_pool = ctx.enter_context(tc.tile_pool(name="const", bufs=1))
            prep_pool = ctx.enter_context(tc.tile_pool(name="prep", bufs=1))
            big_pool = ctx.enter_context(tc.tile_pool(name="big", bufs=1))
            dst_pool = ctx.enter_context(tc.tile_pool(name="dst", bufs=2))
            fold_pool = ctx.enter_context(tc.tile_pool(name="fold", bufs=2))
            out_pool = ctx.enter_context(tc.tile_pool(name="outp", bufs=1))

            # ---------- constants ----------
            ones_y = const_pool.tile([128, yf], fp16, tag="ones_y")
            nc.vector.memset(ones_y[:], 1.0)
            ident = const_pool.tile([128, 128], fp16, tag="ident")
            make_identity(nc, ident[:])

            # ---------- feature prep: x ----------
            craw_x = prep_pool.tile([128, 3 * xf], fp32, tag="craw_x")
            nc.sync.dma_start(
                craw_x[:], x_h.ap().rearrange("(p f) d -> p (f d)", p=128))
            cx = prep_pool.tile([128, 3 * xf], fp32, tag="cx")
            craw_x3 = craw_x[:].rearrange("p (f d) -> p d f", d=3)
            for d in range(3):
                nc.vector.tensor_copy(cx[:, d * xf:(d + 1) * xf],
                                      craw_x3[:, d:d + 1, :])
            n2x = prep_pool.tile([128, xf], fp32, tag="n2x")
            tmpx = prep_pool.tile([128, xf], fp32, tag="tmpx")
            nc.vector.tensor_tensor(n2x[:], cx[:, 0:xf], cx[:, 0:xf], op=OP.mult)
            nc.vector.tensor_tensor(tmpx[:], cx[:, xf:2 * xf], cx[:, xf:2 * xf], op=OP.mult)
            nc.vector.tensor_tensor(n2x[:], n2x[:], tmpx[:], op=OP.add)
            nc.vector.tensor_tensor(tmpx[:], cx[:, 2 * xf:3 * xf], cx[:, 2 * xf:3 * xf], op=OP.mult)
            nc.vector.tensor_tensor(n2x[:], n2x[:], tmpx[:], op=OP.add)
            n2xh = prep_pool.tile([128, xf], fp16, tag="n2xh")
            n2xh32 = prep_pool.tile([128, xf], fp32, tag="n2xh32")
            n2xl = prep_pool.tile([128, xf], fp16, tag="n2xl")
            nc.vector.tensor_copy(n2xh[:], n2x[:])
            nc.scalar.copy(n2xh32[:], n2xh[:])
            nc.vector.tensor_tensor(n2xl[:], n2x[:], n2xh32[:], op=OP.subtract)
            # a = -2x, then hi/lo split
            ax = prep_pool.tile([128, 3 * xf], fp32, tag="ax")
            nc.vector.tensor_scalar_mul(ax[:], cx[:], -2.0)
            axh = prep_pool.tile([128, 3 * xf], fp16, tag="axh")
            axh32 = prep_pool.tile([128, 3 * xf], fp32, tag="axh32")
            axl = prep_pool.tile([128, 3 * xf], fp16, tag="axl")
            nc.vector.tensor_copy(axh[:], ax[:])
            nc.scalar.copy(axh32[:], axh[:])
            nc.vector.tensor_tensor(axl[:], ax[:], axh32[:], op=OP.subtract)

            # ---------- feature prep: y ----------
            # cy[p, d*yf + f] = y[p*yf + f, d]
            # one contiguous DMA (fast), then de-interleave xyz on the DVE -
            # the 4B/12B strided DRAM read pattern costs ~14us per plane.
            craw_y = prep_pool.tile([128, 3 * yf], fp32, tag="craw_y")
            nc.sync.dma_start(
                craw_y[:], y_h.ap().rearrange("(p f) d -> p (f d)", p=128))
            cy = prep_pool.tile([128, 3 * yf], fp32, tag="cy")
            craw_y3 = craw_y[:].rearrange("p (f d) -> p d f", d=3)
            for d in range(3):
                nc.vector.tensor_copy(cy[:, d * yf:(d + 1) * yf],
                                      craw_y3[:, d:d + 1, :])
            n2y = prep_pool.tile([128, yf], fp32, tag="n2y")
            tmpy = prep_pool.tile([128, yf], fp32, tag="tmpy")
            nc.vector.tensor_tensor(n2y[:], cy[:, 0:yf], cy[:, 0:yf], op=OP.mult)
            nc.vector.tensor_tensor(tmpy[:], cy[:, yf:2 * yf], cy[:, yf:2 * yf], op=OP.mult)
            nc.vector.tensor_tensor(n2y[:], n2y[:], tmpy[:], op=OP.add)
            nc.vector.tensor_tensor(tmpy[:], cy[:, 2 * yf:3 * yf], cy[:, 2 * yf:3 * yf], op=OP.mult)
            nc.vector.tensor_tensor(n2y[:], n2y[:], tmpy[:], op=OP.add)
            # hi/lo split of n2y
            n2yh = prep_pool.tile([128, yf], fp16, tag="n2yh")
            n2yh32 = prep_pool.tile([128, yf], fp32, tag="n2yh32")
            n2yl = prep_pool.tile([128, yf], fp16, tag="n2yl")
            nc.vector.tensor_copy(n2yh[:], n2y[:])
            nc.scalar.copy(n2yh32[:], n2yh[:])
            nc.vector.tensor_tensor(n2yl[:], n2y[:], n2yh32[:], op=OP.subtract)
            # hi/lo split of y coords (all 3 at once)
            yh = prep_pool.tile([128, 3 * yf], fp16, tag="yh")
            yh32 = prep_pool.tile([128, 3 * yf], fp32, tag="yh32")
            yl = prep_pool.tile([128, 3 * yf], fp16, tag="yl")
            nc.vector.tensor_copy(yh[:], cy[:])
            nc.scalar.copy(yh32[:], yh[:])
            nc.vector.tensor_tensor(yl[:], cy[:], yh32[:], op=OP.subtract)

            # ---------- assemble K x N operand tiles ----------
            # pairing per K row r:  phi[r] . psi[r]
            #  r0 : 1      * |y|2_h     r1 : 1      * |y|2_l
            #  r2 : |x|2_h * 1          r3 : |x|2_l * 1
            #  r4..6  : axh_d * yh_d    r7..9  : axh_d * yl_d
            #  r10..12: axl_d * yh_d    r13..15: axl_d * yl_d
            # Round-trip through DRAM scratch: the SBUF->DRAM writes keep the
            # [128, f] layout (768B/partition descriptors), and each psi/phi
            # row read becomes one small 2D strided DRAM read - far cheaper
            # than a [128-partition gather] -> [1 partition] SBUF-SBUF DMA.
            # For the 4x PE row tiling, row k of PE group c lives on SBUF
            # partition 32c+k: phi rows are replicated to all 4 bases, psi
            # rows are split by column chunk (group c gets chunk c).
            dram_pool = ctx.enter_context(
                tc.tile_pool(name="dscr", bufs=1, space="DRAM"))
            # per-plane staging tiles: the later row reads become single
            # contiguous 32KB DRAM reads instead of 128-segment gathers.
            d_yh = [dram_pool.tile([128, yf], fp16, tag=f"d_yh{d}",
                                   name=f"d_yh{d}") for d in range(3)]
            d_yl = [dram_pool.tile([128, yf], fp16, tag=f"d_yl{d}",
                                   name=f"d_yl{d}") for d in range(3)]
            d_n2yh = dram_pool.tile([128, yf], fp16, tag="d_n2yh")
            d_n2yl = dram_pool.tile([128, yf], fp16, tag="d_n2yl")
            d_ones = dram_pool.tile([128, yf], fp16, tag="d_ones")
            d_xh = [dram_pool.tile([128, xf], fp16, tag=f"d_xh{d}",
                                   name=f"d_xh{d}") for d in range(3)]
            d_xl = [dram_pool.tile([128, xf], fp16, tag=f"d_xl{d}",
                                   name=f"d_xl{d}") for d in range(3)]
            d_n2xh = dram_pool.tile([128, xf], fp16, tag="d_n2xh")
            d_n2xl = dram_pool.tile([128, xf], fp16, tag="d_n2xl")
            nc.sync.dma_start(d_ones[:], ones_y[:])
            for d in range(3):
                nc.sync.dma_start(d_xh[d][:], axh[:, d * xf:(d + 1) * xf])
                nc.gpsimd.dma_start(d_xl[d][:], axl[:, d * xf:(d + 1) * xf])
            nc.scalar.dma_start(d_n2xh[:], n2xh[:])
            nc.scalar.dma_start(d_n2xl[:], n2xl[:])
            for d in range(3):
                nc.sync.dma_start(d_yh[d][:], yh[:, d * yf:(d + 1) * yf])
                nc.gpsimd.dma_start(d_yl[d][:], yl[:, d * yf:(d + 1) * yf])
            nc.scalar.dma_start(d_n2yh[:], n2yh[:])
            nc.scalar.dma_start(d_n2yl[:], n2yl[:])

            # phi_rep[32c + k, :] = phi row k (same for all c)
            # psi_rep[32c + k, :] = psi row k, global columns [c*4096, +4096)
            # Assemble both at base 0 first (one strided-DRAM read per row),
            # then replicate/shift to bases 32/64/96 with a few fat
            # SBUF->SBUF DMAs (16 partitions x contiguous bytes each).
            phi_rep = big_pool.tile([128, rp], fp16, tag="phi_rep")
            psi_mst = big_pool.tile([128, ny], fp16, tag="psi_mst")
            psi_rep = big_pool.tile([128, chunk], fp16, tag="psi_rep")

            _eng = [nc.sync, nc.gpsimd, nc.scalar]
            _rr = [0]

            def dma(dst, src):
                e = _eng[_rr[0] % len(_eng)]
                _rr[0] += 1
                e.dma_start(dst, src)

            def flat(t):       # [128, f] dram tile -> full linear row
                return t[:, :].rearrange("p f -> (p f)")

            ones_rp = d_ones[0:rp // yf, :].rearrange("p f -> (p f)")

            def ph(k):
                return phi_rep[k:k + 1, :]

            def ps(k):
                return psi_mst[k:k + 1, :]

            dma(ph(0), ones_rp)
            dma(ph(1), ones_rp)
            dma(ph(2), flat(d_n2xh))
            dma(ph(3), flat(d_n2xl))
            for d in range(3):
                dma(ph(4 + d), flat(d_xh[d]))
                dma(ph(7 + d), flat(d_xh[d]))
                dma(ph(10 + d), flat(d_xl[d]))
                dma(ph(13 + d), flat(d_xl[d]))

            dma(ps(0), flat(d_n2yh))
            dma(ps(1), flat(d_n2yl))
            dma(ps(2), flat(d_ones))
            dma(ps(3), flat(d_ones))
            for d in range(3):
                dma(ps(4 + d), flat(d_yh[d]))
                dma(ps(7 + d), flat(d_yl[d]))
                dma(ps(10 + d), flat(d_yh[d]))
                dma(ps(13 + d), flat(d_yl[d]))

            # replicate phi to bases 32/64/96; shift psi chunks c>=1 there.
            # group 0 reads psi_mst / phi_rep base 0 directly.
            for c in range(1, n_grp):
                nc.sync.dma_start(phi_rep[32 * c:32 * c + K, :],
                                  phi_rep[0:K, :])
                nc.gpsimd.dma_start(
                    psi_rep[32 * c:32 * c + K, :],
                    psi_mst[0:K, c * chunk:(c + 1) * chunk])

            # ---------- main loop ----------
            # Per x-tile: 8 drain steps r; step r is PE row group c=r%4,
            # chunk-half h=r//4, i.e. dst cols [r*2048, +2048) = global cols
            # [c*4096 + h*2048, +2048)  (a fixed column permutation - min and
            # mean are permutation invariant, and all cores use the same one).
            colacc = big_pool.tile([128, ny], fp16, tag="colacc")
            rmin = out_pool.tile([128, xt_n], fp32, tag="rmin")

            with tc.tile_pool(name="psum", bufs=2, space="PSUM") as psum_pool:
                for xt in range(xt_n):
                    dst = colacc if xt == 0 else dst_pool.tile(
                        [128, ny], fp16, tag="dst")
                    fold = fold_pool.tile([128, ny // 2], fp16, tag="fold")
                    for t in range(ndr):
                        # psum tile t: column c*512+j of the tile holds
                        # global column c*4096 + t*512 + j -> the 4 matmuls
                        # hit 4 different PE row groups and run concurrently.
                        pt = psum_pool.tile([128, DR], fp32, tag="pt")
                        for c in range(n_grp):
                            b = 32 * c
                            w = phi_rep[b:b + K, xt * 128:(xt + 1) * 128]
                            src_ps = psi_mst if c == 0 else psi_rep
                            rhs = src_ps[b:b + K, t * 512:(t + 1) * 512]
                            nc.tensor.matmul(
                                pt[:, c * 512:(c + 1) * 512],
                                w, rhs,
                                start=True, stop=True,
                                tile_position=(b, 0),
                            )
                        sl = slice(t * DR, (t + 1) * DR)
                        nc.scalar.copy(dst[:, sl], pt[:])
                    # col-min accumulate (full width; chunked on the last
                    # x-tile so the epilogue can start under it)
                    if xt == xt_n - 1:
                        for cch in range(4):
                            sl = slice(cch * ny // 4, (cch + 1) * ny // 4)
                            nc.vector.tensor_tensor(
                                colacc[:, sl], colacc[:, sl], dst[:, sl],
                                op=OP.min)
                    elif xt > 0:
                        nc.vector.tensor_tensor(
                            colacc[:], colacc[:], dst[:], op=OP.min)
                    # row-min: TT-min halving tree (fold1..4) + 1x reduce.
                    nc.vector.tensor_tensor(
                        fold[:], dst[:, :ny // 2], dst[:, ny // 2:],
                        op=OP.min)
                    w2 = ny // 4
                    while w2 >= 256:
                        nc.vector.tensor_tensor(
                            fold[:, :w2], fold[:, :w2], fold[:, w2:2 * w2],
                            op=OP.min)
                        w2 //= 2
                    nc.vector.tensor_reduce(
                        rmin[:, xt:xt + 1], fold[:, :256], axis=AX.X,
                        op=OP.min)

            # ---------- epilogue: partition-min of colacc via PE transpose ----------
            colmin16 = out_pool.tile([128, yf], fp16, tag="colmin16")
            with tc.tile_pool(name="psumT", bufs=2, space="PSUM") as psumt_pool:
                bb = 16  # transposed blocks per batch
                nb = yf // bb
                for b in range(nb):
                    ptile = psumt_pool.tile([128, bb * 128], fp16, tag="ptile")
                    for q in range(bb):
                        blk = b * bb + q
                        nc.tensor.transpose(
                            ptile[:, q * 128:(q + 1) * 128],
                            colacc[:, blk * 128:(blk + 1) * 128],
                            ident[:],
                        )
                    nc.vector.tensor_reduce(
                        colmin16[:, b * bb:(b + 1) * bb],
                        ptile[:].rearrange("p (a f) -> p a f", a=bb),
                        axis=AX.X, op=OP.min,
                    )

            # ---------- clamp + sqrt + store ----------
            colmin32 = out_pool.tile([128, yf], fp32, tag="colmin32")
            nc.vector.tensor_scalar_max(colmin32[:], colmin16[:], 0.0)
            colout = out_pool.tile([128, yf], fp32, tag="colout")
            nc.scalar.activation(colout[:], colmin32[:], ACT.Sqrt)
            nc.sync.dma_start(col_h.ap()[:, :], colout[:])

            rclamp = out_pool.tile([128, xt_n], fp32, tag="rclamp")
            nc.vector.tensor_scalar_max(rclamp[:], rmin[:], 0.0)
            rowout = out_pool.tile([128, xt_n], fp32, tag="rowout")
            nc.scalar.activation(rowout[:], rclamp[:], ACT.Sqrt)
            nc.sync.dma_start(row_h.ap()[:, :], rowout[:])

    nc.compile()
    return nc


_CACHED = None


def _get_module():
    global _CACHED
    if _CACHED is None:
        _CACHED = build_module()
    return _CACHED


def run_on_hw(nc, in_maps, **kw):
    from concourse.bass_utils import run_bass_kernel_spmd
    return run_bass_kernel_spmd(nc, in_maps, core_ids=list(range(N_CORES)), **kw)


def _postprocess(results):
    rowcat = np.concatenate(
        [results[c]["row_out"].T.reshape(-1) for c in range(N_CORES)])
    colmin = np.stack(
        [results[c]["col_out"].T.reshape(-1) for c in range(N_CORES)]).min(axis=0)
    loss = rowcat.mean(dtype=np.float64) + colmin.mean(dtype=np.float64)
    return np.asarray(loss, dtype=np.float32)


def kernel(pred_corners, gt_corners):
    x = np.ascontiguousarray(np.asarray(pred_corners, dtype=np.float32).reshape(-1, 3))
    y = np.ascontiguousarray(np.asarray(gt_corners, dtype=np.float32).reshape(-1, 3))
    assert x.shape == (NX, 3) and y.shape == (NY, 3)
    nc = _get_module()
    in_maps = [
        {"x_shard": x[c * RP:(c + 1) * RP], "y_full": y} for c in range(N_CORES)
    ]
    res = run_on_hw(nc, in_maps)
    return _postprocess(res.results)


# revision 18
# speedup vs baseline: 1.0589x; 1.0002x over previous
"""Chamfer loss kernel for Trainium2, 8 NeuronCores.

Strategy (sharding_hint): row-block the 16384x16384 distance matrix.
Core c owns x rows [c*2048, (c+1)*2048) (x = flattened pred corners) and
all 16384 y points (flattened gt corners). Each core computes, on device:
  - d2[n, m] = |x_n|^2 + |y_m|^2 - 2 x.y  for its row block, via a K=16
    fp16 matmul using hi/lo fp16 splits of the operands (~fp32 accuracy).
    The PE array is row-tiled 4x (tile_position=(32c,0)): row group c
    handles the 4096-column chunk c, so 4 matmuls stream concurrently in
    the 128x128 array (K=16 uses only 16 of 32 rows per group).
  - per-PSUM-tile drain to SBUF fp16 on the scalar (ACT) engine,
  - row mins via a single fused tensor_tensor_reduce (halves fold + min
    accumulate) per x-tile on the DVE,
  - a running column min (TT-min) per x-tile on the DVE,
  - column partition-min via PE transposes + tensor_reduce epilogue.
Host glue: shard x, gather per-core row mins, all-reduce(min) the partial
column mins across the 8 cores, then mean both and add - the loss.
"""

import sys
import numpy as np

if "/opt/trn_rl_repo" not in sys.path:
    sys.path.insert(0, "/opt/trn_rl_repo")

# ---- hardcoded problem geometry (from the task spec) ----
N_CORES = 8
NX = 16384          # total x points (2048 boxes * 8 corners)
NY = 16384          # total y points
RP = NX // N_CORES  # 2048 x rows per core
XT = RP // 128      # 16 x tiles of 128 rows
K = 16              # contraction rows of the split matmul
NGRP = 4            # PE row groups (4x tiling); group c owns cols [c*4096, +4096)
CHUNK = NY // NGRP  # 4096 cols per row group
DR = 2048           # columns per drain tile (4 PSUM banks)
NDR = NY // DR      # 8 drain steps per x-tile


def build_module(rp=RP, ny=NY, n_grp=NGRP, use_ttr=False, gp_folds=False):
    # NOTE: gp_folds=True (nc.gpsimd.tensor_tensor fp16 min) also crashes on
    # TRN2 hardware via this runner; row-tree stays on the DVE.
    # NOTE: use_ttr=True (InstTensorTensorReduce) crashes on TRN2 hardware
    # (and is 1x-rate anyway); keep the TT-min halving tree.
    """Build + compile the per-core Bass module. Returns the Bacc object.

    n_grp: number of PE row groups (4 = full row tiling, 1 = none).
    use_ttr: row-min via fused tensor_tensor_reduce vs TT-min tree.
    """
    from contextlib import ExitStack

    import concourse.tile as tile
    from concourse import bacc, mybir
    from concourse.masks import make_identity

    fp32 = mybir.dt.float32
    fp16 = mybir.dt.float16
    AX = mybir.AxisListType
    OP = mybir.AluOpType
    ACT = mybir.ActivationFunctionType

    xt_n = rp // 128
    xf = rp // 128       # free cols per partition for x feature tiles
    yf = ny // 128
    chunk = ny // n_grp  # columns per PE row group
    ndr = ny // DR       # drain steps per x-tile

    nc = bacc.Bacc("TRN2", target_bir_lowering=False, debug=False,
                   num_devices=N_CORES)
    x_h = nc.dram_tensor("x_shard", [rp, 3], fp32, kind="ExternalInput")
    y_h = nc.dram_tensor("y_full", [ny, 3], fp32, kind="ExternalInput")
    row_h = nc.dram_tensor("row_out", [128, xt_n], fp32, kind="ExternalOutput")
    col_h = nc.dram_tensor("col_out", [128, yf], fp32, kind="ExternalOutput")

    with tile.TileContext(nc) as tc:
        with ExitStack() as ctx:
            const_pool = ctx.enter_context(tc.tile_pool(name="const", bufs=1))
            prep_pool = ctx.enter_context(tc.tile_pool(name="prep", bufs=1))
            big_pool = ctx.enter_context(tc.tile_pool(name="big", bufs=1))
            dst_pool = ctx.enter_context(tc.tile_pool(name="dst", bufs=2))
            fold_pool = ctx.enter_context(tc.tile_pool(name="fold", bufs=2))
            out_pool = ctx.enter_context(tc.tile_pool(name="outp", bufs=1))

            # ---------- constants ----------
            ones_y = const_pool.tile([128, yf], fp16, tag="ones_y")
            nc.vector.memset(ones_y[:], 1.0)
            ident = const_pool.tile([128, 128], fp16, tag="ident")
            make_identity(nc, ident[:])

            # ---------- feature prep: y ----------
            # cy[p, d*yf + f] = y[p*yf + f, d]
            # one contiguous DMA (fast), then de-interleave xyz on the DVE -
            # the 4B/12B strided DRAM read pattern costs ~14us per plane.
            craw_y = prep_pool.tile([128, 3 * yf], fp32, tag="craw_y")
            nc.sync.dma_start(
                craw_y[:], y_h.ap().rearrange("(p f) d -> p (f d)", p=128))
            cy = prep_pool.tile([128, 3 * yf], fp32, tag="cy")
            craw_y3 = craw_y[:].rearrange("p (f d) -> p d f", d=3)
            for d in range(3):
                nc.vector.tensor_copy(cy[:, d * yf:(d + 1) * yf],
                                      craw_y3[:, d:d + 1, :])
            n2y = prep_pool.tile([128, yf], fp32, tag="n2y")
            tmpy = prep_pool.tile([128, yf], fp32, tag="tmpy")
            nc.vector.tensor_tensor(n2y[:], cy[:, 0:yf], cy[:, 0:yf], op=OP.mult)
            nc.vector.tensor_tensor(tmpy[:], cy[:, yf:2 * yf], cy[:, yf:2 * yf], op=OP.mult)
            nc.vector.tensor_tensor(n2y[:], n2y[:], tmpy[:], op=OP.add)
            nc.vector.tensor_tensor(tmpy[:], cy[:, 2 * yf:3 * yf], cy[:, 2 * yf:3 * yf], op=OP.mult)
            nc.vector.tensor_tensor(n2y[:], n2y[:], tmpy[:], op=OP.add)
            # hi/lo split of n2y
            n2yh = prep_pool.tile([128, yf], fp16, tag="n2yh")
            n2yh32 = prep_pool.tile([128, yf], fp32, tag="n2yh32")
            n2yl = prep_pool.tile([128, yf], fp16, tag="n2yl")
            nc.vector.tensor_copy(n2yh[:], n2y[:])
            nc.scalar.copy(n2yh32[:], n2yh[:])
            nc.vector.tensor_tensor(n2yl[:], n2y[:], n2yh32[:], op=OP.subtract)
            # hi/lo split of y coords (all 3 at once)
            yh = prep_pool.tile([128, 3 * yf], fp16, tag="yh")
            yh32 = prep_pool.tile([128, 3 * yf], fp32, tag="yh32")
            yl = prep_pool.tile([128, 3 * yf], fp16, tag="yl")
            nc.vector.tensor_copy(yh[:], cy[:])
            nc.scalar.copy(yh32[:], yh[:])
            nc.vector.tensor_tensor(yl[:], cy[:], yh32[:], op=OP.subtract)

            # ---------- feature prep: x ----------
            craw_x = prep_pool.tile([128, 3 * xf], fp32, tag="craw_x")
            nc.sync.dma_start(
                craw_x[:], x_h.ap().rearrange("(p f) d -> p (f d)", p=128))
            cx = prep_pool.tile([128, 3 * xf], fp32, tag="cx")
            craw_x3 = craw_x[:].rearrange("p (f d) -> p d f", d=3)
            for d in range(3):
                nc.vector.tensor_copy(cx[:, d * xf:(d + 1) * xf],
                                      craw_x3[:, d:d + 1, :])
            n2x = prep_pool.tile([128, xf], fp32, tag="n2x")
            tmpx = prep_pool.tile([128, xf], fp32, tag="tmpx")
            nc.vector.tensor_tensor(n2x[:], cx[:, 0:xf], cx[:, 0:xf], op=OP.mult)
            nc.vector.tensor_tensor(tmpx[:], cx[:, xf:2 * xf], cx[:, xf:2 * xf], op=OP.mult)
            nc.vector.tensor_tensor(n2x[:], n2x[:], tmpx[:], op=OP.add)
            nc.vector.tensor_tensor(tmpx[:], cx[:, 2 * xf:3 * xf], cx[:, 2 * xf:3 * xf], op=OP.mult)
            nc.vector.tensor_tensor(n2x[:], n2x[:], tmpx[:], op=OP.add)
            n2xh = prep_pool.tile([128, xf], fp16, tag="n2xh")
            n2xh32 = prep_pool.tile([128, xf], fp32, tag="n2xh32")
            n2xl = prep_pool.tile([128, xf], fp16, tag="n2xl")
            nc.vector.tensor_copy(n2xh[:], n2x[:])
            nc.scalar.copy(n2xh32[:], n2xh[:])
            nc.vector.tensor_tensor(n2xl[:], n2x[:], n2xh32[:], op=OP.subtract)
            # a = -2x, then hi/lo split
            ax = prep_pool.tile([128, 3 * xf], fp32, tag="ax")
            nc.vector.tensor_scalar_mul(ax[:], cx[:], -2.0)
            axh = prep_pool.tile([128, 3 * xf], fp16, tag="axh")
            axh32 = prep_pool.tile([128, 3 * xf], fp32, tag="axh32")
            axl = prep_pool.tile([128, 3 * xf], fp16, tag="axl")
            nc.vector.tensor_copy(axh[:], ax[:])
            nc.scalar.copy(axh32[:], axh[:])
            nc.vector.tensor_tensor(axl[:], ax[:], axh32[:], op=OP.subtract)

            # ---------- assemble K x N operand tiles ----------
            # pairing per K row r:  phi[r] . psi[r]
            #  r0 : 1      * |y|2_h     r1 : 1      * |y|2_l
            #  r2 : |x|2_h * 1          r3 : |x|2_l * 1
            #  r4..6  : axh_d * yh_d    r7..9  : axh_d * yl_d
            #  r10..12: axl_d * yh_d    r13..15: axl_d * yl_d
            # Round-trip through DRAM scratch: the SBUF->DRAM writes keep the
            # [128, f] layout (768B/partition descriptors), and each psi/phi
            # row read becomes one small 2D strided DRAM read - far cheaper
            # than a [128-partition gather] -> [1 partition] SBUF-SBUF DMA.
            # For the 4x PE row tiling, row k of PE group c lives on SBUF
            # partition 32c+k: phi rows are replicated to all 4 bases, psi
            # rows are split by column chunk (group c gets chunk c).
            dram_pool = ctx.enter_context(
                tc.tile_pool(name="dscr", bufs=1, space="DRAM"))
            # per-plane staging tiles: the later row reads become single
            # contiguous 32KB DRAM reads instead of 128-segment gathers.
            d_yh = [dram_pool.tile([128, yf], fp16, tag=f"d_yh{d}",
                                   name=f"d_yh{d}") for d in range(3)]
            d_yl = [dram_pool.tile([128, yf], fp16, tag=f"d_yl{d}",
                                   name=f"d_yl{d}") for d in range(3)]
            d_n2yh = dram_pool.tile([128, yf], fp16, tag="d_n2yh")
            d_n2yl = dram_pool.tile([128, yf], fp16, tag="d_n2yl")
            d_ones = dram_pool.tile([128, yf], fp16, tag="d_ones")
            d_xh = [dram_pool.tile([128, xf], fp16, tag=f"d_xh{d}",
                                   name=f"d_xh{d}") for d in range(3)]
            d_xl = [dram_pool.tile([128, xf], fp16, tag=f"d_xl{d}",
                                   name=f"d_xl{d}") for d in range(3)]
            d_n2xh = dram_pool.tile([128, xf], fp16, tag="d_n2xh")
            d_n2xl = dram_pool.tile([128, xf], fp16, tag="d_n2xl")
            for d in range(3):
                nc.sync.dma_start(d_yh[d][:], yh[:, d * yf:(d + 1) * yf])
                nc.gpsimd.dma_start(d_yl[d][:], yl[:, d * yf:(d + 1) * yf])
            nc.scalar.dma_start(d_n2yh[:], n2yh[:])
            nc.scalar.dma_start(d_n2yl[:], n2yl[:])
            nc.sync.dma_start(d_ones[:], ones_y[:])
            for d in range(3):
                nc.sync.dma_start(d_xh[d][:], axh[:, d * xf:(d + 1) * xf])
                nc.gpsimd.dma_start(d_xl[d][:], axl[:, d * xf:(d + 1) * xf])
            nc.scalar.dma_start(d_n2xh[:], n2xh[:])
            nc.scalar.dma_start(d_n2xl[:], n2xl[:])

            # phi_rep[32c + k, :] = phi row k (same for all c)
            # psi_rep[32c + k, :] = psi row k, global columns [c*4096, +4096)
            # Assemble both at base 0 first (one strided-DRAM read per row),
            # then replicate/shift to bases 32/64/96 with a few fat
            # SBUF->SBUF DMAs (16 partitions x contiguous bytes each).
            phi_rep = big_pool.tile([128, rp], fp16, tag="phi_rep")
            psi_mst = big_pool.tile([128, ny], fp16, tag="psi_mst")
            psi_rep = big_pool.tile([128, chunk], fp16, tag="psi_rep")

            _eng = [nc.sync, nc.gpsimd, nc.scalar]
            _rr = [0]

            def dma(dst, src):
                e = _eng[_rr[0] % len(_eng)]
                _rr[0] += 1
                e.dma_start(dst, src)

            def flat(t):       # [128, f] dram tile -> full linear row
                return t[:, :].rearrange("p f -> (p f)")

            ones_rp = d_ones[0:rp // yf, :].rearrange("p f -> (p f)")

            def ph(k):
                return phi_rep[k:k + 1, :]

            def ps(k):
                return psi_mst[k:k + 1, :]

            # interleave psi (y-side, gates the shifts) with phi (x-side)
            dma(ps(0), flat(d_n2yh))
            dma(ph(0), ones_rp)
            dma(ps(1), flat(d_n2yl))
            dma(ph(1), ones_rp)
            dma(ps(2), flat(d_ones))
            dma(ph(2), flat(d_n2xh))
            dma(ps(3), flat(d_ones))
            dma(ph(3), flat(d_n2xl))
            for d in range(3):
                dma(ps(4 + d), flat(d_yh[d]))
                dma(ph(4 + d), flat(d_xh[d]))
                dma(ps(7 + d), flat(d_yl[d]))
                dma(ph(7 + d), flat(d_xh[d]))
                dma(ps(10 + d), flat(d_yh[d]))
                dma(ph(10 + d), flat(d_xl[d]))
                dma(ps(13 + d), flat(d_yl[d]))
                dma(ph(13 + d), flat(d_xl[d]))

            # replicate phi to bases 32/64/96; shift psi chunks c>=1 there.
            # group 0 reads psi_mst / phi_rep base 0 directly.
            for c in range(1, n_grp):
                nc.sync.dma_start(phi_rep[32 * c:32 * c + K, :],
                                  phi_rep[0:K, :])
                nc.gpsimd.dma_start(
                    psi_rep[32 * c:32 * c + K, :],
                    psi_mst[0:K, c * chunk:(c + 1) * chunk])

            # ---------- main loop ----------
            # Per x-tile: 8 drain steps r; step r is PE row group c=r%4,
            # chunk-half h=r//4, i.e. dst cols [r*2048, +2048) = global cols
            # [c*4096 + h*2048, +2048)  (a fixed column permutation - min and
            # mean are permutation invariant, and all cores use the same one).
            colacc = big_pool.tile([128, ny], fp16, tag="colacc")
            rmin = out_pool.tile([128, xt_n], fp32, tag="rmin")

            with tc.tile_pool(name="psum", bufs=2, space="PSUM") as psum_pool:
                for xt in range(xt_n):
                    dst = colacc if xt == 0 else dst_pool.tile(
                        [128, ny], fp16, tag="dst")
                    fold = fold_pool.tile([128, ny // 2], fp16, tag="fold")
                    for t in range(ndr):
                        # psum tile t: column c*512+j of the tile holds
                        # global column c*4096 + t*512 + j -> the 4 matmuls
                        # hit 4 different PE row groups and run concurrently.
                        pt = psum_pool.tile([128, DR], fp32, tag="pt")
                        for c in range(n_grp):
                            b = 32 * c
                            w = phi_rep[b:b + K, xt * 128:(xt + 1) * 128]
                            src_ps = psi_mst if c == 0 else psi_rep
                            rhs = src_ps[b:b + K, t * 512:(t + 1) * 512]
                            nc.tensor.matmul(
                                pt[:, c * 512:(c + 1) * 512],
                                w, rhs,
                                start=True, stop=True,
                                tile_position=(b, 0),
                            )
                        sl = slice(t * DR, (t + 1) * DR)
                        nc.scalar.copy(dst[:, sl], pt[:])
                    # col-min accumulate (full width; chunked on the last
                    # x-tile so the epilogue can start under it)
                    if xt == xt_n - 1:
                        for cch in range(4):
                            sl = slice(cch * ny // 4, (cch + 1) * ny // 4)
                            nc.vector.tensor_tensor(
                                colacc[:, sl], colacc[:, sl], dst[:, sl],
                                op=OP.min)
                    elif xt > 0:
                        nc.vector.tensor_tensor(
                            colacc[:], colacc[:], dst[:], op=OP.min)
                    # row-min: TT-min halving tree (fold1..4) + 1x reduce.
                    nc.vector.tensor_tensor(
                        fold[:], dst[:, :ny // 2], dst[:, ny // 2:],
                        op=OP.min)
                    w2 = ny // 4
                    while w2 >= 256:
                        nc.vector.tensor_tensor(
                            fold[:, :w2], fold[:, :w2], fold[:, w2:2 * w2],
                            op=OP.min)
                        w2 //= 2
                    nc.vector.tensor_reduce(
                        rmin[:, xt:xt + 1], fold[:, :256], axis=AX.X,
                        op=OP.min)

            # ---------- epilogue: partition-min of colacc via PE transpose ----------
            colmin16 = out_pool.tile([128, yf], fp16, tag="colmin16")
            with tc.tile_pool(name="psumT", bufs=2, space="PSUM") as psumt_pool:
                bb = 16  # transposed blocks per batch
                nb = yf // bb
                for b in range(nb):
                    ptile = psumt_pool.tile([128, bb * 128], fp16, tag="ptile")
                    for q in range(bb):
                        blk = b * bb + q
                        nc.tensor.transpose(
                            ptile[:, q * 128:(q + 1) * 128],
                            colacc[:, blk * 128:(blk + 1) * 128],
                            ident[:],
                        )
                    nc.vector.tensor_reduce(
                        colmin16[:, b * bb:(b + 1) * bb],
                        ptile[:].rearrange("p (a f) -> p a f", a=bb),
                        axis=AX.X, op=OP.min,
                    )

            # ---------- clamp + sqrt + store ----------
            colmin32 = out_pool.tile([128, yf], fp32, tag="colmin32")
            nc.vector.tensor_scalar_max(colmin32[:], colmin16[:], 0.0)
            colout = out_pool.tile([128, yf], fp32, tag="colout")
            nc.scalar.activation(colout[:], colmin32[:], ACT.Sqrt)
            nc.sync.dma_start(col_h.ap()[:, :], colout[:])

            rclamp = out_pool.tile([128, xt_n], fp32, tag="rclamp")
            nc.vector.tensor_scalar_max(rclamp[:], rmin[:], 0.0)
            rowout = out_pool.tile([128, xt_n], fp32, tag="rowout")
            nc.scalar.activation(rowout[:], rclamp[:], ACT.Sqrt)
            nc.sync.dma_start(row_h.ap()[:, :], rowout[:])

    nc.compile()
    return nc


_CACHED = None


def _get_module():
    global _CACHED
    if _CACHED is None:
        _CACHED = build_module()
    return _CACHED


def run_on_hw(nc, in_maps, **kw):
    from concourse.bass_utils import run_bass_kernel_spmd
    return run_bass_kernel_spmd(nc, in_maps, core_ids=list(range(N_CORES)), **kw)


def _postprocess(results):
    rowcat = np.concatenate(
        [results[c]["row_out"].T.reshape(-1) for c in range(N_CORES)])
    colmin = np.stack(
        [results[c]["col_out"].T.reshape(-1) for c in range(N_CORES)]).min(axis=0)
    loss = rowcat.mean(dtype=np.float64) + colmin.mean(dtype=np.float64)
    return np.asarray(loss, dtype=np.float32)


def kernel(pred_corners, gt_corners):
    x = np.ascontiguousarray(np.asarray(pred_corners, dtype=np.float32).reshape(-1, 3))
    y = np.ascontiguousarray(np.asarray(gt_corners, dtype=np.float32).reshape(-1, 3))
    assert x.shape == (NX, 3) and y.shape == (NY, 3)
    nc = _get_module()
    in_maps = [
        {"x_shard": x[c * RP:(c + 1) * RP], "y_full": y} for c in range(N_CORES)
    ]
    res = run_on_hw(nc, in_maps)
    return _postprocess(res.results)
